# revision 6
# baseline (speedup 1.0000x reference)
"""ConvLSTM forward (ot gate only) as a Trainium2 Bass kernel.

The reference module returns only
    ot = sigmoid(conv(x, W8)+b8 + conv(H0, W9)+b9 + conv(C0, W10)+b10 + bgate[3])
(the it/ft/Ct computations are dead code).  The H0/C0 terms are
batch-independent, so each core computes a residual
    R = conv(H0, W9) + conv(C0, W10) + bgate[3] + (b8+b9+b10)
once and then per image computes  sigmoid(conv(x_i, W8) + R).

Sharding: data-parallel over batch.  32 images / 8 cores = 4 images per
core; H0/C0/bgate[3] and the tiny conv weights are replicated.

The 3x3 'same' conv runs on the TensorEngine: the image is processed in
bands of 126 output rows; the input tile for a band carries one halo row
above and below (128 partitions).  The three vertical taps are folded
into tridiagonal 128x128 "band" lhsT matrices (one per horizontal offset
dx), so
    psum[m, c] += sum_k Band_dx[k, m] * x[k, c+dx]
accumulates the full 3x3 conv over three matmuls per column chunk;
horizontal taps come from shifted column APs of the same SBUF tile, and
column/row image edges fall out of restricted AP ranges (zero padding).
The residual R is added through a fourth (identity lhsT) matmul into the
same PSUM accumulation group, and the sigmoid runs on the Scalar engine
reading PSUM directly.
"""

import os
from contextlib import ExitStack

import numpy as np

import concourse.bass as bass
import concourse.bacc as bacc
import concourse.mybir as mybir
from concourse import tile
from concourse.bass_utils import run_bass_kernel_spmd

F32 = mybir.dt.float32
F32R = mybir.dt.float32r

N_CORES = 8
TR = 126  # output rows per band (input tiles overlap by 2 rows)
CH = 512  # psum column chunk (max fp32 moving free dim)


def _build_bands(w_list):
    """Band (tridiagonal) lhsT matrices for the vertical conv taps.

    Returns [19, 128, 128]: for conv ci in (0,1,2) and dy slot dyi in (0,1,2):
      bands[ci*3+dyi..] packs, per horizontal dx slot, a banded matrix.
    Layout: index b = ci*3 + dxi for interior tiles (input partition k =
    image row r0-1+k, psum partition m = image row r0+m, so k = m+dyi),
    b = 9 + ci*3 + dxi for the top tile (k = image row k, so k = m+dyi-1),
    b = 18 is the identity (for the R-add matmul).
    """
    bands = np.zeros((19, 128, 128), np.float32)
    for ci, w in enumerate(w_list):
        for dxi in range(3):
            Bn = bands[ci * 3 + dxi]
            Bt = bands[9 + ci * 3 + dxi]
            for m in range(128):
                for dyi in range(3):
                    k = m + dyi  # interior tiles
                    if 0 <= k < 128:
                        Bn[k, m] = w[dyi, dxi]
                    k = m + dyi - 1  # top tile
                    if 0 <= k < 128:
                        Bt[k, m] = w[dyi, dxi]
    bands[18] = np.eye(128, dtype=np.float32)
    return bands


def _build_nc(B, H, W, use_f32r=True):
    """Build the per-core Bass program (SPMD: same program on all cores)."""
    T = -(-H // TR)  # number of output bands
    DT = F32R if use_f32r else F32
    nc = bacc.Bacc(None, target_bir_lowering=False, debug=False)

    # xs/h0/c0 are padded with one zero column on each side (host-side)
    xs = nc.dram_tensor("xs", [B, H, W + 2], DT, kind="ExternalInput")
    h0 = nc.dram_tensor("h0", [H, W + 2], DT, kind="ExternalInput")
    c0 = nc.dram_tensor("c0", [H, W + 2], DT, kind="ExternalInput")
    bg3 = nc.dram_tensor("bg3", [H, W], F32, kind="ExternalInput")
    bandsD = nc.dram_tensor("bands", [19, 128, 128], DT, kind="ExternalInput")
    biasD = nc.dram_tensor("biasv", [128, 1], F32, kind="ExternalInput")
    outD = nc.dram_tensor("out", [B, H, W], F32, kind="ExternalOutput")

    def mm(ap):
        return ap

    def geom(i):
        """(r0, nrows, lo, K, top) for band i.

        Output rows [r0, r0+nrows) live at psum partitions [0, nrows).
        Input rows [lo, lo+K) live at tile partitions [0, K).
        """
        r0 = TR * i
        nrows = min(TR, H - r0)
        if i == 0:
            return r0, nrows, 0, min(TR + 1, H), True
        lo = r0 - 1
        return r0, nrows, lo, min(H - lo, TR + 2), False

    # column chunks
    chunks = []
    c = 0
    while c < W:
        chunks.append((c, min(c + CH, W)))
        c += CH

    with tile.TileContext(nc) as tc, ExitStack() as ctx:
        const_pool = ctx.enter_context(tc.tile_pool(name="const", bufs=1))
        rpool = ctx.enter_context(tc.tile_pool(name="rsb", bufs=1))
        xpool = ctx.enter_context(tc.tile_pool(name="xin", bufs=4))
        hpool = ctx.enter_context(tc.tile_pool(name="hin", bufs=2))
        opool = ctx.enter_context(tc.tile_pool(name="osb", bufs=4))
        rpsum = ctx.enter_context(tc.tile_pool(name="rps", bufs=2, space="PSUM"))
        ipsum = ctx.enter_context(tc.tile_pool(name="ips", bufs=2, space="PSUM"))

        # constants
        bsb = const_pool.tile([128, 19, 128], DT)
        nc.sync.dma_start(out=bsb[:], in_=bandsD.rearrange("b p f -> p b f"))
        bias_sb = const_pool.tile([128, 1], F32)
        nc.sync.dma_start(out=bias_sb[:], in_=biasD[:])

        # residual R: R_sb[m, i, :] = R row TR*i + m, m in [0, nrows_i)
        R_sb = rpool.tile([128, T, W], DT)

        def band_ap(ci, dxi, top, K, M):
            b = (9 if top else 0) + ci * 3 + dxi
            return bsb[0:K, b, 0:M]

        def conv_mms(psum, xt, ci, i, first_per_chunk, last_per_chunk):
            """Emit the 3 dx matmuls for conv `ci` on (column-padded) tile xt.

            Image col c of the psum reads padded cols c+dxi for dxi in
            0..2 (the tile carries zero columns at 0 and W+1), so every
            matmul is a full, even-width chunk (fp32r ISA requirement).
            """
            r0, nrows, lo, K, top = geom(i)
            M = nrows
            for (ca, cb) in chunks:
                for dxi in (1, 0, 2):
                    nc.tensor.matmul(
                        psum[0:M, ca:cb],
                        mm(band_ap(ci, dxi, top, K, M)),
                        mm(xt[0:K, ca + dxi:cb + dxi]),
                        start=(first_per_chunk and dxi == 1),
                        stop=(last_per_chunk and dxi == 2),
                    )

        # ---- residual R ----
        for j in range(T):
            r0, nrows, lo, K, top = geom(j)
            ht = hpool.tile([128, W + 2], DT, tag="ht")
            ct = hpool.tile([128, W + 2], DT, tag="ct")
            bgt = hpool.tile([128, W], F32, tag="bgt")
            nc.sync.dma_start(out=ht[0:K, :], in_=h0[lo:lo + K, :])
            nc.sync.dma_start(out=ct[0:K, :], in_=c0[lo:lo + K, :])
            nc.sync.dma_start(out=bgt[0:nrows, :], in_=bg3[r0:r0 + nrows, :])
            psum = rpsum.tile([128, W], F32, tag="rps")
            conv_mms(psum, ht, 1, j, True, False)
            conv_mms(psum, ct, 2, j, False, True)
            # R rows = conv psum + bias + bgate3
            nc.vector.scalar_tensor_tensor(
                out=R_sb[0:nrows, j, :],
                in0=psum[0:nrows, :],
                scalar=bias_sb[0:nrows, :],
                in1=bgt[0:nrows, :],
                op0=mybir.AluOpType.add,
                op1=mybir.AluOpType.add,
            )

        # ---- images ----
        for img in range(B):
            for i in range(T):
                r0, nrows, lo, K, top = geom(i)
                xt = xpool.tile([128, W + 2], DT, tag="xt")
                nc.sync.dma_start(out=xt[0:K, :], in_=xs[img, lo:lo + K, :])
                psum = ipsum.tile([128, W], F32, tag="ips")
                conv_mms(psum, xt, 0, i, True, False)
                # R add via identity matmul (completes the accumulation group)
                for ki, (ca, cb) in enumerate(chunks):
                    nc.tensor.matmul(
                        psum[0:nrows, ca:cb],
                        mm(bsb[0:nrows, 18, 0:nrows]),
                        mm(R_sb[0:nrows, i, ca:cb]),
                        start=False, stop=(ki == len(chunks) - 1),
                    )
                ot = opool.tile([128, W], F32, tag="ot")
                nc.scalar.activation(
                    ot[0:nrows, :], psum[0:nrows, :],
                    mybir.ActivationFunctionType.Sigmoid,
                )
                nc.sync.dma_start(out=outD[img, r0:r0 + nrows, :], in_=ot[0:nrows, :])

    nc.compile()
    return nc


_NC_CACHE = {}


def _get_nc(B, H, W, use_f32r=True):
    key = (B, H, W, use_f32r)
    if key not in _NC_CACHE:
        _NC_CACHE[key] = _build_nc(B, H, W, use_f32r)
    return _NC_CACHE[key]


def _make_inmaps(x, H0, C0, Wconv, bconv, bgate, n_cores):
    B = x.shape[0]
    per = B // n_cores
    H, W = x.shape[2], x.shape[3]
    pad = ((0, 0), (0, 0), (1, 1))
    x2 = np.pad(np.asarray(x, np.float32).reshape(B, H, W), pad)
    Wc = np.asarray(Wconv, np.float32)
    bands = _build_bands([Wc[8, 0, 0], Wc[9, 0, 0], Wc[10, 0, 0]])
    bc = np.asarray(bconv, np.float32)
    bias = np.full((128, 1), bc[8] + bc[9] + bc[10], np.float32)
    h0 = np.pad(np.asarray(H0, np.float32)[0, 0], ((0, 0), (1, 1)))
    c0 = np.pad(np.asarray(C0, np.float32)[0, 0], ((0, 0), (1, 1)))
    bg3 = np.ascontiguousarray(np.asarray(bgate, np.float32)[3])
    return [
        {
            "xs": np.ascontiguousarray(x2[c * per:(c + 1) * per]),
            "h0": h0, "c0": c0, "bg3": bg3,
            "bands": bands, "biasv": bias,
        }
        for c in range(n_cores)
    ]


def kernel(x, H0, C0, Wconv, bconv, bgate):
    B, _, H, W = x.shape
    per = B // N_CORES
    use_f32r = os.environ.get("CONV_NO_F32R", "") != "1"
    nc = _get_nc(per, H, W, use_f32r)
    in_maps = _make_inmaps(x, H0, C0, Wconv, bconv, bgate, N_CORES)
    trace = os.environ.get("CONV_TRACE", "") == "1"
    res = run_bass_kernel_spmd(nc, in_maps, list(range(N_CORES)), trace=trace)
    if trace:
        kernel.last_exec_time_ns = res.exec_time_ns
        kernel.last_results = res
    out = np.concatenate([r["out"] for r in res.results], axis=0)
    return out.reshape(B, 1, H, W).astype(np.float32)


# revision 8
# speedup vs baseline: 1.9189x; 1.9189x over previous
"""ConvLSTM forward (ot gate only) as a Trainium2 Bass kernel.

The reference module returns only
    ot = sigmoid(conv(x, W8)+b8 + conv(H0, W9)+b9 + conv(C0, W10)+b10 + bgate[3])
(the it/ft/Ct computations are dead code).  The H0/C0 terms are
batch-independent, so each core computes a residual band
    R = conv(H0, W9) + conv(C0, W10) + bgate[3] + (b8+b9+b10)
once and then per image computes  sigmoid(conv(x_i, W8) + R).

Sharding: spatial over H.  Core c produces output rows [126c, 126c+126)
of all 32 images (x slabs carry a 1-row halo), so H0/C0/bgate loads and
the R conv are sharded 8-ways instead of replicated.  The 16-row
remainder (1024 = 8*126 + 16) is batch-sharded: each core computes the
tail rows of its 4 "home" images.

The 3x3 'same' conv runs on the TensorEngine.  Inputs are zero-padded
by one row/column on each side (host-side), so a band of 126 output
rows reads a 128-row input tile and every core uses the same
tridiagonal 128x128 "band" lhsT matrices:
    psum[m, c] += sum_k Band_dx[k, m] * x[k, c+dx]
accumulates the full 3x3 conv over three matmuls per 512-column chunk
(horizontal taps are shifted column APs of the same SBUF tile; matmuls
run as float32r for full PE rate).  The residual add runs on the Vector
engine (PSUM + R -> SBUF) and the sigmoid on the Scalar engine.
"""

import os
from contextlib import ExitStack

import numpy as np

import concourse.bass as bass
import concourse.bacc as bacc
import concourse.mybir as mybir
from concourse import tile
from concourse.bass_utils import run_bass_kernel_spmd

F32 = mybir.dt.float32
F32R = mybir.dt.float32r

N_CORES = 8
TR = 126  # output rows per band (input tiles carry a 1-row halo each side)
CH = 512  # psum column chunk (max fp32 moving free dim)


def _build_bands(w_list):
    """Tridiagonal lhsT matrices for the vertical conv taps.

    Input tile partition k holds padded image row r0+k (= image row
    r0+k-1); psum partition m holds output image row r0+m.  The tap at
    vertical offset dy reads input partition k = m+dy+1 = m+dyi, so
    Band[m+dyi, m] = w[dyi, dxi].
    Returns [9, 128, 128] for conv ci in (0,1,2) x dxi in (0,1,2),
    pre-transposed to [128, 9, 128] (partition-major) for a linear DMA.
    """
    bands = np.zeros((9, 128, 128), np.float32)
    for ci, w in enumerate(w_list):
        for dxi in range(3):
            B = bands[ci * 3 + dxi]
            for m in range(128):
                for dyi in range(3):
                    k = m + dyi
                    if k < 128:
                        B[k, m] = w[dyi, dxi]
    return np.ascontiguousarray(bands.transpose(1, 0, 2))  # [128, 9, 128]


def _build_nc(B, H, W, use_f32r=True, n_cores=N_CORES):
    """Per-core Bass program (SPMD: same program, different data).

    B: total images (each core sees all of them for its main band).
    Main band: TR output rows; tail: TT = H - 7*TR... computed from H.
    """
    DT = F32R if use_f32r else F32
    TT = H - TR * n_cores  # tail rows (batch-sharded), 16 for H=1024
    BH = B // n_cores  # home images per core
    Wp = W + 2
    nc = bacc.Bacc(None, target_bir_lowering=False, debug=False)

    # Main-band inputs: padded rows [126c, 126c+128) of every image.
    xm = nc.dram_tensor("xm", [B, 128, Wp], DT, kind="ExternalInput")
    h0m = nc.dram_tensor("h0m", [128, Wp], DT, kind="ExternalInput")
    c0m = nc.dram_tensor("c0m", [128, Wp], DT, kind="ExternalInput")
    bg3m = nc.dram_tensor("bg3m", [TR, W], F32, kind="ExternalInput")
    # Tail inputs: padded rows [H-TT, H+2) of the BH home images.
    KT = TT + 2
    xt4 = nc.dram_tensor("xt4", [BH, KT, Wp], DT, kind="ExternalInput")
    h0t = nc.dram_tensor("h0t", [KT, Wp], DT, kind="ExternalInput")
    c0t = nc.dram_tensor("c0t", [KT, Wp], DT, kind="ExternalInput")
    bg3t = nc.dram_tensor("bg3t", [TT, W], F32, kind="ExternalInput")

    bandsD = nc.dram_tensor("bands", [128, 9, 128], DT, kind="ExternalInput")
    biasD = nc.dram_tensor("biasv", [128, 1], F32, kind="ExternalInput")

    outm = nc.dram_tensor("outm", [B, TR, W], F32, kind="ExternalOutput")
    outt = nc.dram_tensor("outt", [BH, TT, W], F32, kind="ExternalOutput")

    chunks = []
    c = 0
    while c < W:
        chunks.append((c, min(c + CH, W)))
        c += CH

    with tile.TileContext(nc) as tc, ExitStack() as ctx:
        const_pool = ctx.enter_context(tc.tile_pool(name="const", bufs=1))
        rpool = ctx.enter_context(tc.tile_pool(name="rsb", bufs=1))
        xpool = ctx.enter_context(tc.tile_pool(name="xin", bufs=4))
        hpool = ctx.enter_context(tc.tile_pool(name="hin", bufs=1))
        opool = ctx.enter_context(tc.tile_pool(name="osb", bufs=4))
        rpsum = ctx.enter_context(tc.tile_pool(name="rps", bufs=1, space="PSUM"))
        ipsum = ctx.enter_context(tc.tile_pool(name="ips", bufs=3, space="PSUM"))

        # constants
        bsb = const_pool.tile([128, 9, 128], DT)
        nc.sync.dma_start(out=bsb[:], in_=bandsD[:])
        bias_sb = const_pool.tile([128, 1], F32)
        nc.sync.dma_start(out=bias_sb[:], in_=biasD[:])

        def conv_mms(psum, xt, ci, K, M, first, last):
            """3 dx matmuls per column chunk for conv ci on tile xt."""
            for (ca, cb) in chunks:
                for dxi in (1, 0, 2):
                    nc.tensor.matmul(
                        psum[0:M, ca:cb],
                        bsb[0:K, ci * 3 + dxi, 0:M],
                        xt[0:K, ca + dxi:cb + dxi],
                        start=(first and dxi == 1),
                        stop=(last and dxi == 2),
                    )

        def residual(h_src, c_src, bg_src, K, M, tag):
            """R band = conv(H0,W9) + conv(C0,W10) + bias + bgate3."""
            ht = hpool.tile([128, Wp], DT, tag="ht" + tag)
            ct = hpool.tile([128, Wp], DT, tag="ct" + tag)
            bgt = hpool.tile([128, W], F32, tag="bgt" + tag)
            nc.sync.dma_start(out=ht[0:K, :], in_=h_src)
            nc.sync.dma_start(out=ct[0:K, :], in_=c_src)
            nc.sync.dma_start(out=bgt[0:M, :], in_=bg_src)
            psum = rpsum.tile([128, W], F32, tag="rps")
            conv_mms(psum, ht, 1, K, M, True, False)
            conv_mms(psum, ct, 2, K, M, False, True)
            R = rpool.tile([128, W], F32, tag="R" + tag)
            nc.vector.scalar_tensor_tensor(
                out=R[0:M, :], in0=psum[0:M, :], scalar=bias_sb[0:M, :],
                in1=bgt[0:M, :],
                op0=mybir.AluOpType.add, op1=mybir.AluOpType.add,
            )
            return R

        def image_band(x_src, out_dst, R, K, M):
            xt = xpool.tile([128, Wp], DT, tag="xt")
            nc.sync.dma_start(out=xt[0:K, :], in_=x_src)
            psum = ipsum.tile([128, W], F32, tag="ips")
            conv_mms(psum, xt, 0, K, M, True, True)
            ot = opool.tile([128, W], F32, tag="ot")
            nc.vector.tensor_add(out=ot[0:M, :], in0=psum[0:M, :], in1=R[0:M, :])
            nc.scalar.activation(
                ot[0:M, :], ot[0:M, :], mybir.ActivationFunctionType.Sigmoid,
            )
            nc.sync.dma_start(out=out_dst, in_=ot[0:M, :])

        # ---- main band: R, then all B images ----
        Rm = residual(h0m[:], c0m[:], bg3m[:], 128, TR, "m")
        # ---- tail residual up front too (loads are tiny) ----
        Rt = residual(h0t[:], c0t[:], bg3t[:], KT, TT, "t")

        for img in range(B):
            image_band(xm[img], outm[img], Rm, 128, TR)
            if img % (B // BH) == 0:  # interleave a home-image tail every 8th
                g = img // (B // BH)
                if g < BH:
                    image_band(xt4[g], outt[g], Rt, KT, TT)

    nc.compile()
    return nc


_NC_CACHE = {}


def _get_nc(B, H, W, use_f32r=True, n_cores=N_CORES):
    key = (B, H, W, use_f32r, n_cores)
    if key not in _NC_CACHE:
        _NC_CACHE[key] = _build_nc(B, H, W, use_f32r, n_cores)
    return _NC_CACHE[key]


def _make_inmaps(x, H0, C0, Wconv, bconv, bgate, n_cores):
    B = x.shape[0]
    H, W = x.shape[2], x.shape[3]
    BH = B // n_cores
    TT = H - TR * n_cores
    KT = TT + 2

    xp = np.pad(np.asarray(x, np.float32).reshape(B, H, W), ((0, 0), (1, 1), (1, 1)))
    h0p = np.pad(np.asarray(H0, np.float32)[0, 0], 1)
    c0p = np.pad(np.asarray(C0, np.float32)[0, 0], 1)
    bg3 = np.ascontiguousarray(np.asarray(bgate, np.float32)[3])
    Wc = np.asarray(Wconv, np.float32)
    bands = _build_bands([Wc[8, 0, 0], Wc[9, 0, 0], Wc[10, 0, 0]])
    bc = np.asarray(bconv, np.float32)
    bias = np.full((128, 1), bc[8] + bc[9] + bc[10], np.float32)

    maps = []
    for c in range(n_cores):
        r0 = TR * c
        maps.append({
            "xm": np.ascontiguousarray(xp[:, r0:r0 + 128, :]),
            "h0m": np.ascontiguousarray(h0p[r0:r0 + 128, :]),
            "c0m": np.ascontiguousarray(c0p[r0:r0 + 128, :]),
            "bg3m": np.ascontiguousarray(bg3[r0:r0 + TR, :]),
            "xt4": np.ascontiguousarray(xp[c * BH:(c + 1) * BH, H - TT:H + 2, :]),
            "h0t": np.ascontiguousarray(h0p[H - TT:H + 2, :]),
            "c0t": np.ascontiguousarray(c0p[H - TT:H + 2, :]),
            "bg3t": np.ascontiguousarray(bg3[H - TT:, :]),
            "bands": bands, "biasv": bias,
        })
    return maps


def kernel(x, H0, C0, Wconv, bconv, bgate):
    B, _, H, W = x.shape
    use_f32r = os.environ.get("CONV_NO_F32R", "") != "1"
    nc = _get_nc(B, H, W, use_f32r)
    in_maps = _make_inmaps(x, H0, C0, Wconv, bconv, bgate, N_CORES)
    trace = os.environ.get("CONV_TRACE", "") == "1"
    res = run_bass_kernel_spmd(nc, in_maps, list(range(N_CORES)), trace=trace)
    if trace:
        kernel.last_exec_time_ns = res.exec_time_ns
        kernel.last_results = res

    out = np.empty((B, H, W), np.float32)
    BH = B // N_CORES
    TT = H - TR * N_CORES
    for c in range(N_CORES):
        r0 = TR * c
        out[:, r0:r0 + TR, :] = res.results[c]["outm"]
        if TT:
            out[c * BH:(c + 1) * BH, H - TT:, :] = res.results[c]["outt"]
    return out.reshape(B, 1, H, W)


# revision 9
# speedup vs baseline: 2.0441x; 1.0653x over previous
"""ConvLSTM forward (ot gate only) as a Trainium2 Bass kernel.

The reference module returns only
    ot = sigmoid(conv(x, W8)+b8 + conv(H0, W9)+b9 + conv(C0, W10)+b10 + bgate[3])
(the it/ft/Ct computations are dead code).  The H0/C0 terms are
batch-independent, so each core computes a residual band
    R = conv(H0, W9) + conv(C0, W10) + bgate[3] + (b8+b9+b10)
once and then per image computes  sigmoid(conv(x_i, W8) + R).

Sharding: spatial over H.  Core c produces output rows [126c, 126c+126)
of all 32 images (x slabs carry a 1-row halo), so H0/C0/bgate loads and
the R conv are sharded 8-ways instead of replicated.  The 16-row
remainder (1024 = 8*126 + 16) is batch-sharded: each core computes the
tail rows of its 4 "home" images.

The 3x3 'same' conv runs on the TensorEngine.  Inputs are zero-padded
by one row/column on each side (host-side), so a band of 126 output
rows reads a 128-row input tile and every core uses the same
tridiagonal 128x128 "band" lhsT matrices:
    psum[m, c] += sum_k Band_dx[k, m] * x[k, c+dx]
accumulates the full 3x3 conv over three matmuls per 512-column chunk
(horizontal taps are shifted column APs of the same SBUF tile; matmuls
run as float32r for full PE rate).  The residual add runs on the Vector
engine (PSUM + R -> SBUF) and the sigmoid on the Scalar engine.
"""

import os
from contextlib import ExitStack

import numpy as np

import concourse.bass as bass
import concourse.bacc as bacc
import concourse.mybir as mybir
from concourse import tile
from concourse.bass_utils import run_bass_kernel_spmd

F32 = mybir.dt.float32
F32R = mybir.dt.float32r

N_CORES = 8
TR = 126  # output rows per band (input tiles carry a 1-row halo each side)
CH = 512  # psum column chunk (max fp32 moving free dim)


def _build_bands(w_list):
    """Tridiagonal lhsT matrices for the vertical conv taps.

    Input tile partition k holds padded image row r0+k (= image row
    r0+k-1); psum partition m holds output image row r0+m.  The tap at
    vertical offset dy reads input partition k = m+dy+1 = m+dyi, so
    Band[m+dyi, m] = w[dyi, dxi].
    Returns [9, 128, 128] for conv ci in (0,1,2) x dxi in (0,1,2),
    pre-transposed to [128, 9, 128] (partition-major) for a linear DMA.
    """
    bands = np.zeros((9, 128, 128), np.float32)
    for ci, w in enumerate(w_list):
        for dxi in range(3):
            B = bands[ci * 3 + dxi]
            for m in range(128):
                for dyi in range(3):
                    k = m + dyi
                    if k < 128:
                        B[k, m] = w[dyi, dxi]
    return np.ascontiguousarray(bands.transpose(1, 0, 2))  # [128, 9, 128]


def _build_nc(B, H, W, use_f32r=True, n_cores=N_CORES):
    """Per-core Bass program (SPMD: same program, different data).

    B: total images (each core sees all of them for its main band).
    Main band: TR output rows; tail: TT = H - 7*TR... computed from H.
    """
    DT = F32R if use_f32r else F32
    TT = H - TR * n_cores  # tail rows (batch-sharded), 16 for H=1024
    BH = B // n_cores  # home images per core
    Wp = W + 2
    nc = bacc.Bacc(None, target_bir_lowering=False, debug=False)

    # Main-band inputs: padded rows [126c, 126c+128) of every image.
    xm = nc.dram_tensor("xm", [B, 128, Wp], DT, kind="ExternalInput")
    h0m = nc.dram_tensor("h0m", [128, Wp], DT, kind="ExternalInput")
    c0m = nc.dram_tensor("c0m", [128, Wp], DT, kind="ExternalInput")
    bg3m = nc.dram_tensor("bg3m", [TR, W], F32, kind="ExternalInput")
    # Tail inputs: padded rows [H-TT, H+2) of the BH home images.
    KT = TT + 2
    xt4 = nc.dram_tensor("xt4", [BH, KT, Wp], DT, kind="ExternalInput")
    h0t = nc.dram_tensor("h0t", [KT, Wp], DT, kind="ExternalInput")
    c0t = nc.dram_tensor("c0t", [KT, Wp], DT, kind="ExternalInput")
    bg3t = nc.dram_tensor("bg3t", [TT, W], F32, kind="ExternalInput")

    bandsD = nc.dram_tensor("bands", [128, 9, 128], DT, kind="ExternalInput")
    biasD = nc.dram_tensor("biasv", [128, 1], F32, kind="ExternalInput")

    outm = nc.dram_tensor("outm", [B, TR, W], F32, kind="ExternalOutput")
    outt = nc.dram_tensor("outt", [BH, TT, W], F32, kind="ExternalOutput")

    chunks = []
    c = 0
    while c < W:
        chunks.append((c, min(c + CH, W)))
        c += CH

    with tile.TileContext(nc) as tc, ExitStack() as ctx:
        const_pool = ctx.enter_context(tc.tile_pool(name="const", bufs=1))
        rpool = ctx.enter_context(tc.tile_pool(name="rsb", bufs=1))
        xpool = ctx.enter_context(tc.tile_pool(name="xin", bufs=8))
        hpool = ctx.enter_context(tc.tile_pool(name="hin", bufs=1))
        opool = ctx.enter_context(tc.tile_pool(name="osb", bufs=6))
        ipsum = ctx.enter_context(tc.tile_pool(name="ips", bufs=4, space="PSUM"))

        # constants
        bsb = const_pool.tile([128, 9, 128], DT)
        nc.sync.dma_start(out=bsb[:], in_=bandsD[:])
        bias_sb = const_pool.tile([128, 1], F32)
        nc.sync.dma_start(out=bias_sb[:], in_=biasD[:])

        def conv_mms(psum, xt, ci, K, M, first, last):
            """3 dx matmuls per column chunk for conv ci on tile xt."""
            for (ca, cb) in chunks:
                for dxi in (1, 0, 2):
                    nc.tensor.matmul(
                        psum[0:M, ca:cb],
                        bsb[0:K, ci * 3 + dxi, 0:M],
                        xt[0:K, ca + dxi:cb + dxi],
                        start=(first and dxi == 1),
                        stop=(last and dxi == 2),
                    )

        def residual(h_src, c_src, bg_src, K, M, tag):
            """R band = conv(H0,W9) + conv(C0,W10) + bias + bgate3."""
            ht = hpool.tile([128, Wp], DT, tag="ht" + tag)
            ct = hpool.tile([128, Wp], DT, tag="ct" + tag)
            bgt = hpool.tile([128, W], F32, tag="bgt" + tag)
            nc.sync.dma_start(out=ht[0:K, :], in_=h_src)
            nc.sync.dma_start(out=ct[0:K, :], in_=c_src)
            nc.sync.dma_start(out=bgt[0:M, :], in_=bg_src)
            psum = ipsum.tile([128, W], F32, tag="ips")
            conv_mms(psum, ht, 1, K, M, True, False)
            conv_mms(psum, ct, 2, K, M, False, True)
            R = rpool.tile([128, W], F32, tag="R" + tag)
            nc.vector.scalar_tensor_tensor(
                out=R[0:M, :], in0=psum[0:M, :], scalar=bias_sb[0:M, :],
                in1=bgt[0:M, :],
                op0=mybir.AluOpType.add, op1=mybir.AluOpType.add,
            )
            return R

        def image_band(x_src, out_dst, R, K, M):
            xt = xpool.tile([128, Wp], DT, tag="xt")
            nc.sync.dma_start(out=xt[0:K, :], in_=x_src)
            psum = ipsum.tile([128, W], F32, tag="ips")
            conv_mms(psum, xt, 0, K, M, True, True)
            ot = opool.tile([128, W], F32, tag="ot")
            nc.vector.tensor_add(out=ot[0:M, :], in0=psum[0:M, :], in1=R[0:M, :])
            nc.scalar.activation(
                ot[0:M, :], ot[0:M, :], mybir.ActivationFunctionType.Sigmoid,
            )
            nc.sync.dma_start(out=out_dst, in_=ot[0:M, :])

        # ---- main band: R, then all B images ----
        Rm = residual(h0m[:], c0m[:], bg3m[:], 128, TR, "m")
        # ---- tail residual up front too (loads are tiny) ----
        Rt = residual(h0t[:], c0t[:], bg3t[:], KT, TT, "t")

        for img in range(B):
            image_band(xm[img], outm[img], Rm, 128, TR)
            if img % (B // BH) == 0:  # interleave a home-image tail every 8th
                g = img // (B // BH)
                if g < BH:
                    image_band(xt4[g], outt[g], Rt, KT, TT)

    nc.compile()
    return nc


_NC_CACHE = {}


def _get_nc(B, H, W, use_f32r=True, n_cores=N_CORES):
    key = (B, H, W, use_f32r, n_cores)
    if key not in _NC_CACHE:
        _NC_CACHE[key] = _build_nc(B, H, W, use_f32r, n_cores)
    return _NC_CACHE[key]


def _make_inmaps(x, H0, C0, Wconv, bconv, bgate, n_cores):
    B = x.shape[0]
    H, W = x.shape[2], x.shape[3]
    BH = B // n_cores
    TT = H - TR * n_cores
    KT = TT + 2

    xp = np.pad(np.asarray(x, np.float32).reshape(B, H, W), ((0, 0), (1, 1), (1, 1)))
    h0p = np.pad(np.asarray(H0, np.float32)[0, 0], 1)
    c0p = np.pad(np.asarray(C0, np.float32)[0, 0], 1)
    bg3 = np.ascontiguousarray(np.asarray(bgate, np.float32)[3])
    Wc = np.asarray(Wconv, np.float32)
    bands = _build_bands([Wc[8, 0, 0], Wc[9, 0, 0], Wc[10, 0, 0]])
    bc = np.asarray(bconv, np.float32)
    bias = np.full((128, 1), bc[8] + bc[9] + bc[10], np.float32)

    maps = []
    for c in range(n_cores):
        r0 = TR * c
        maps.append({
            "xm": np.ascontiguousarray(xp[:, r0:r0 + 128, :]),
            "h0m": np.ascontiguousarray(h0p[r0:r0 + 128, :]),
            "c0m": np.ascontiguousarray(c0p[r0:r0 + 128, :]),
            "bg3m": np.ascontiguousarray(bg3[r0:r0 + TR, :]),
            "xt4": np.ascontiguousarray(xp[c * BH:(c + 1) * BH, H - TT:H + 2, :]),
            "h0t": np.ascontiguousarray(h0p[H - TT:H + 2, :]),
            "c0t": np.ascontiguousarray(c0p[H - TT:H + 2, :]),
            "bg3t": np.ascontiguousarray(bg3[H - TT:, :]),
            "bands": bands, "biasv": bias,
        })
    return maps


def kernel(x, H0, C0, Wconv, bconv, bgate):
    B, _, H, W = x.shape
    use_f32r = os.environ.get("CONV_NO_F32R", "") != "1"
    nc = _get_nc(B, H, W, use_f32r)
    in_maps = _make_inmaps(x, H0, C0, Wconv, bconv, bgate, N_CORES)
    trace = os.environ.get("CONV_TRACE", "") == "1"
    res = run_bass_kernel_spmd(nc, in_maps, list(range(N_CORES)), trace=trace)
    if trace:
        kernel.last_exec_time_ns = res.exec_time_ns
        kernel.last_results = res

    out = np.empty((B, H, W), np.float32)
    BH = B // N_CORES
    TT = H - TR * N_CORES
    for c in range(N_CORES):
        r0 = TR * c
        out[:, r0:r0 + TR, :] = res.results[c]["outm"]
        if TT:
            out[c * BH:(c + 1) * BH, H - TT:, :] = res.results[c]["outt"]
    return out.reshape(B, 1, H, W)


# revision 10
# speedup vs baseline: 2.5940x; 1.2690x over previous
"""ConvLSTM forward (ot gate only) as a Trainium2 Bass kernel.

The reference module returns only
    ot = sigmoid(conv(x, W8)+b8 + conv(H0, W9)+b9 + conv(C0, W10)+b10 + bgate[3])
(the it/ft/Ct computations are dead code).  The H0/C0 terms are
batch-independent, so each core computes a residual band
    R = conv(H0, W9) + conv(C0, W10) + bgate[3] + (b8+b9+b10)
once and then per image computes  sigmoid(conv(x_i, W8) + R).

Sharding: spatial over H.  Core c produces output rows [126c, 126c+126)
of all 32 images (x slabs carry a 1-row halo), so H0/C0/bgate loads and
the R conv are sharded 8-ways instead of replicated.  The 16-row
remainder (1024 = 8*126 + 16) is batch-sharded: each core computes the
tail rows of its 4 "home" images.

The 3x3 'same' conv runs on the TensorEngine.  Inputs are zero-padded
by one row/column on each side (host-side), so a band of 126 output
rows reads a 128-row input tile and every core uses the same
tridiagonal 128x128 "band" lhsT matrices:
    psum[m, c] += sum_k Band_dx[k, m] * x[k, c+dx]
accumulates the full 3x3 conv over three matmuls per 512-column chunk
(horizontal taps are shifted column APs of the same SBUF tile; matmuls
run as float32r for full PE rate).  The residual add runs on the Vector
engine (PSUM + R -> SBUF) and the sigmoid on the Scalar engine.
"""

import os
from contextlib import ExitStack

import numpy as np

import concourse.bass as bass
import concourse.bacc as bacc
import concourse.mybir as mybir
from concourse import tile
from concourse.bass_utils import run_bass_kernel_spmd

F32 = mybir.dt.float32
F32R = mybir.dt.float32r
F16 = mybir.dt.float16
_DT_MAP = {"f32r": F32R, "fp16": F16, "f32": F32}

N_CORES = 8
TR = 126  # output rows per band (input tiles carry a 1-row halo each side)
CH = 512  # psum column chunk (max fp32 moving free dim)


def _build_bands(w_list):
    """Tridiagonal lhsT matrices for the vertical conv taps.

    Input tile partition k holds padded image row r0+k (= image row
    r0+k-1); psum partition m holds output image row r0+m.  The tap at
    vertical offset dy reads input partition k = m+dy+1 = m+dyi, so
    Band[m+dyi, m] = w[dyi, dxi].
    Returns [9, 128, 128] for conv ci in (0,1,2) x dxi in (0,1,2),
    pre-transposed to [128, 9, 128] (partition-major) for a linear DMA.
    """
    bands = np.zeros((9, 128, 128), np.float32)
    for ci, w in enumerate(w_list):
        for dxi in range(3):
            B = bands[ci * 3 + dxi]
            for m in range(128):
                for dyi in range(3):
                    k = m + dyi
                    if k < 128:
                        B[k, m] = w[dyi, dxi]
    return np.ascontiguousarray(bands.transpose(1, 0, 2))  # [128, 9, 128]


def _build_nc(B, H, W, dt_mode="fp16", n_cores=N_CORES):
    """Per-core Bass program (SPMD: same program, different data).

    B: total images (each core sees all of them for its main band).
    Main band: TR output rows; tail: TT = H - 7*TR... computed from H.
    """
    DT = _DT_MAP[dt_mode]
    TT = H - TR * n_cores  # tail rows (batch-sharded), 16 for H=1024
    BH = B // n_cores  # home images per core
    Wp = W + 2
    nc = bacc.Bacc(None, target_bir_lowering=False, debug=False)

    # Main-band inputs: padded rows [126c, 126c+128) of every image.
    xm = nc.dram_tensor("xm", [B, 128, Wp], DT, kind="ExternalInput")
    h0m = nc.dram_tensor("h0m", [128, Wp], DT, kind="ExternalInput")
    c0m = nc.dram_tensor("c0m", [128, Wp], DT, kind="ExternalInput")
    bg3m = nc.dram_tensor("bg3m", [TR, W], F32, kind="ExternalInput")
    # Tail inputs: padded rows [H-TT, H+2) of the BH home images.
    KT = TT + 2
    xt4 = nc.dram_tensor("xt4", [BH, KT, Wp], DT, kind="ExternalInput")
    h0t = nc.dram_tensor("h0t", [KT, Wp], DT, kind="ExternalInput")
    c0t = nc.dram_tensor("c0t", [KT, Wp], DT, kind="ExternalInput")
    bg3t = nc.dram_tensor("bg3t", [TT, W], F32, kind="ExternalInput")

    bandsD = nc.dram_tensor("bands", [128, 9, 128], DT, kind="ExternalInput")
    biasD = nc.dram_tensor("biasv", [128, 1], F32, kind="ExternalInput")

    outm = nc.dram_tensor("outm", [B, TR, W], F32, kind="ExternalOutput")
    outt = nc.dram_tensor("outt", [BH, TT, W], F32, kind="ExternalOutput")

    chunks = []
    c = 0
    while c < W:
        chunks.append((c, min(c + CH, W)))
        c += CH

    with tile.TileContext(nc) as tc, ExitStack() as ctx:
        const_pool = ctx.enter_context(tc.tile_pool(name="const", bufs=1))
        rpool = ctx.enter_context(tc.tile_pool(name="rsb", bufs=1))
        xpool = ctx.enter_context(tc.tile_pool(name="xin", bufs=8))
        hpool = ctx.enter_context(tc.tile_pool(name="hin", bufs=1))
        opool = ctx.enter_context(tc.tile_pool(name="osb", bufs=6))
        ipsum = ctx.enter_context(tc.tile_pool(name="ips", bufs=4, space="PSUM"))

        # constants
        bsb = const_pool.tile([128, 9, 128], DT)
        nc.sync.dma_start(out=bsb[:], in_=bandsD[:])
        bias_sb = const_pool.tile([128, 1], F32)
        nc.sync.dma_start(out=bias_sb[:], in_=biasD[:])

        def conv_mms(psum, xt, ci, K, M, first, last):
            """3 dx matmuls per column chunk for conv ci on tile xt."""
            for (ca, cb) in chunks:
                for dxi in (1, 0, 2):
                    nc.tensor.matmul(
                        psum[0:M, ca:cb],
                        bsb[0:K, ci * 3 + dxi, 0:M],
                        xt[0:K, ca + dxi:cb + dxi],
                        start=(first and dxi == 1),
                        stop=(last and dxi == 2),
                    )

        def residual(h_src, c_src, bg_src, K, M, tag):
            """R band = conv(H0,W9) + conv(C0,W10) + bias + bgate3."""
            ht = hpool.tile([128, Wp], DT, tag="ht" + tag)
            ct = hpool.tile([128, Wp], DT, tag="ct" + tag)
            bgt = hpool.tile([128, W], F32, tag="bgt" + tag)
            nc.sync.dma_start(out=ht[0:K, :], in_=h_src)
            nc.sync.dma_start(out=ct[0:K, :], in_=c_src)
            nc.sync.dma_start(out=bgt[0:M, :], in_=bg_src)
            psum = ipsum.tile([128, W], F32, tag="ips")
            conv_mms(psum, ht, 1, K, M, True, False)
            conv_mms(psum, ct, 2, K, M, False, True)
            R = rpool.tile([128, W], F32, tag="R" + tag)
            nc.vector.scalar_tensor_tensor(
                out=R[0:M, :], in0=psum[0:M, :], scalar=bias_sb[0:M, :],
                in1=bgt[0:M, :],
                op0=mybir.AluOpType.add, op1=mybir.AluOpType.add,
            )
            return R

        def image_band(x_src, out_dst, R, K, M):
            xt = xpool.tile([128, Wp], DT, tag="xt")
            nc.sync.dma_start(out=xt[0:K, :], in_=x_src)
            psum = ipsum.tile([128, W], F32, tag="ips")
            conv_mms(psum, xt, 0, K, M, True, True)
            ot = opool.tile([128, W], F32, tag="ot")
            nc.vector.tensor_add(out=ot[0:M, :], in0=psum[0:M, :], in1=R[0:M, :])
            nc.scalar.activation(
                ot[0:M, :], ot[0:M, :], mybir.ActivationFunctionType.Sigmoid,
            )
            nc.sync.dma_start(out=out_dst, in_=ot[0:M, :])

        # ---- main band: R, then all B images ----
        Rm = residual(h0m[:], c0m[:], bg3m[:], 128, TR, "m")
        # ---- tail residual up front too (loads are tiny) ----
        Rt = residual(h0t[:], c0t[:], bg3t[:], KT, TT, "t")

        for img in range(B):
            image_band(xm[img], outm[img], Rm, 128, TR)
            if img % (B // BH) == 0:  # interleave a home-image tail every 8th
                g = img // (B // BH)
                if g < BH:
                    image_band(xt4[g], outt[g], Rt, KT, TT)

    nc.compile()
    return nc


_NC_CACHE = {}


def _get_nc(B, H, W, dt_mode="fp16", n_cores=N_CORES):
    key = (B, H, W, dt_mode, n_cores)
    if key not in _NC_CACHE:
        _NC_CACHE[key] = _build_nc(B, H, W, dt_mode, n_cores)
    return _NC_CACHE[key]


def _make_inmaps(x, H0, C0, Wconv, bconv, bgate, n_cores, dt_mode="fp16"):
    B = x.shape[0]
    H, W = x.shape[2], x.shape[3]
    BH = B // n_cores
    TT = H - TR * n_cores
    KT = TT + 2

    ndt = np.float16 if dt_mode == "fp16" else np.float32
    xp = np.pad(np.asarray(x, ndt).reshape(B, H, W), ((0, 0), (1, 1), (1, 1)))
    h0p = np.pad(np.asarray(H0, ndt)[0, 0], 1)
    c0p = np.pad(np.asarray(C0, ndt)[0, 0], 1)
    bg3 = np.ascontiguousarray(np.asarray(bgate, np.float32)[3])
    Wc = np.asarray(Wconv, np.float32)
    bands = _build_bands([Wc[8, 0, 0], Wc[9, 0, 0], Wc[10, 0, 0]]).astype(ndt)
    bc = np.asarray(bconv, np.float32)
    bias = np.full((128, 1), bc[8] + bc[9] + bc[10], np.float32)

    maps = []
    for c in range(n_cores):
        r0 = TR * c
        maps.append({
            "xm": np.ascontiguousarray(xp[:, r0:r0 + 128, :]),
            "h0m": np.ascontiguousarray(h0p[r0:r0 + 128, :]),
            "c0m": np.ascontiguousarray(c0p[r0:r0 + 128, :]),
            "bg3m": np.ascontiguousarray(bg3[r0:r0 + TR, :]),
            "xt4": np.ascontiguousarray(xp[c * BH:(c + 1) * BH, H - TT:H + 2, :]),
            "h0t": np.ascontiguousarray(h0p[H - TT:H + 2, :]),
            "c0t": np.ascontiguousarray(c0p[H - TT:H + 2, :]),
            "bg3t": np.ascontiguousarray(bg3[H - TT:, :]),
            "bands": bands, "biasv": bias,
        })
    return maps


def kernel(x, H0, C0, Wconv, bconv, bgate):
    B, _, H, W = x.shape
    dt_mode = os.environ.get("CONV_DT", "fp16")
    nc = _get_nc(B, H, W, dt_mode)
    in_maps = _make_inmaps(x, H0, C0, Wconv, bconv, bgate, N_CORES, dt_mode)
    trace = os.environ.get("CONV_TRACE", "") == "1"
    res = run_bass_kernel_spmd(nc, in_maps, list(range(N_CORES)), trace=trace)
    if trace:
        kernel.last_exec_time_ns = res.exec_time_ns
        kernel.last_results = res

    out = np.empty((B, H, W), np.float32)
    BH = B // N_CORES
    TT = H - TR * N_CORES
    for c in range(N_CORES):
        r0 = TR * c
        out[:, r0:r0 + TR, :] = res.results[c]["outm"]
        if TT:
            out[c * BH:(c + 1) * BH, H - TT:, :] = res.results[c]["outt"]
    return out.reshape(B, 1, H, W)


# revision 11
# speedup vs baseline: 2.6833x; 1.0344x over previous
"""ConvLSTM forward (ot gate only) as a Trainium2 Bass kernel.

The reference module returns only
    ot = sigmoid(conv(x, W8)+b8 + conv(H0, W9)+b9 + conv(C0, W10)+b10 + bgate[3])
(the it/ft/Ct computations are dead code).  The H0/C0 terms are
batch-independent, so each core computes a residual band
    R = conv(H0, W9) + conv(C0, W10) + bgate[3] + (b8+b9+b10)
once and then per image computes  sigmoid(conv(x_i, W8) + R).

Sharding: spatial over H.  Core c produces output rows [126c, 126c+126)
of all 32 images (x slabs carry a 1-row halo), so H0/C0/bgate loads and
the R conv are sharded 8-ways instead of replicated.  The 16-row
remainder (1024 = 8*126 + 16) is batch-sharded: each core computes the
tail rows of its 4 "home" images.

The 3x3 'same' conv runs on the TensorEngine.  Inputs are zero-padded
by one row/column on each side (host-side), so a band of 126 output
rows reads a 128-row input tile and every core uses the same
tridiagonal 128x128 "band" lhsT matrices:
    psum[m, c] += sum_k Band_dx[k, m] * x[k, c+dx]
accumulates the full 3x3 conv over three matmuls per 512-column chunk
(horizontal taps are shifted column APs of the same SBUF tile; matmuls
run as float32r for full PE rate).  The residual add runs on the Vector
engine (PSUM + R -> SBUF) and the sigmoid on the Scalar engine.
"""

import os
from contextlib import ExitStack

import numpy as np

import concourse.bass as bass
import concourse.bacc as bacc
import concourse.mybir as mybir
from concourse import tile
from concourse.bass_utils import run_bass_kernel_spmd

F32 = mybir.dt.float32
F32R = mybir.dt.float32r
F16 = mybir.dt.float16
_DT_MAP = {"f32r": F32R, "fp16": F16, "f32": F32}

N_CORES = 8
TR = 126  # output rows per band (input tiles carry a 1-row halo each side)
CH = 512  # psum column chunk (max fp32 moving free dim)


def _build_bands(w_list):
    """Tridiagonal lhsT matrices for the vertical conv taps.

    Input tile partition k holds padded image row r0+k (= image row
    r0+k-1); psum partition m holds output image row r0+m.  The tap at
    vertical offset dy reads input partition k = m+dy+1 = m+dyi, so
    Band[m+dyi, m] = w[dyi, dxi].
    Returns [9, 128, 128] for conv ci in (0,1,2) x dxi in (0,1,2),
    pre-transposed to [128, 9, 128] (partition-major) for a linear DMA.
    """
    bands = np.zeros((9, 128, 128), np.float32)
    for ci, w in enumerate(w_list):
        for dxi in range(3):
            B = bands[ci * 3 + dxi]
            for m in range(128):
                for dyi in range(3):
                    k = m + dyi
                    if k < 128:
                        B[k, m] = w[dyi, dxi]
    out = np.zeros((10, 128, 128), np.float32)
    out[:9] = bands
    out[9] = np.eye(128, dtype=np.float32)
    return np.ascontiguousarray(out.transpose(1, 0, 2))  # [128, 10, 128]


def _build_nc(B, H, W, dt_mode="fp16", n_cores=N_CORES):
    """Per-core Bass program (SPMD: same program, different data).

    B: total images (each core sees all of them for its main band).
    Main band: TR output rows; tail: TT = H - 7*TR... computed from H.
    """
    DT = _DT_MAP[dt_mode]
    TT = H - TR * n_cores  # tail rows (batch-sharded), 16 for H=1024
    BH = B // n_cores  # home images per core
    Wp = W + 2
    nc = bacc.Bacc(None, target_bir_lowering=False, debug=False)

    # Main-band inputs: padded rows [126c, 126c+128) of every image.
    xm = nc.dram_tensor("xm", [B, 128, Wp], DT, kind="ExternalInput")
    h0m = nc.dram_tensor("h0m", [128, Wp], DT, kind="ExternalInput")
    c0m = nc.dram_tensor("c0m", [128, Wp], DT, kind="ExternalInput")
    bg3m = nc.dram_tensor("bg3m", [TR, W], F32, kind="ExternalInput")
    # Tail inputs: padded rows [H-TT, H+2) of the BH home images.
    KT = TT + 2
    xt4 = nc.dram_tensor("xt4", [BH, KT, Wp], DT, kind="ExternalInput")
    h0t = nc.dram_tensor("h0t", [KT, Wp], DT, kind="ExternalInput")
    c0t = nc.dram_tensor("c0t", [KT, Wp], DT, kind="ExternalInput")
    bg3t = nc.dram_tensor("bg3t", [TT, W], F32, kind="ExternalInput")

    bandsD = nc.dram_tensor("bands", [128, 10, 128], DT, kind="ExternalInput")
    biasD = nc.dram_tensor("biasv", [128, 1], F32, kind="ExternalInput")

    outm = nc.dram_tensor("outm", [B, TR, W], F32, kind="ExternalOutput")
    outt = nc.dram_tensor("outt", [BH, TT, W], F32, kind="ExternalOutput")

    chunks = []
    c = 0
    while c < W:
        chunks.append((c, min(c + CH, W)))
        c += CH

    with tile.TileContext(nc) as tc, ExitStack() as ctx:
        const_pool = ctx.enter_context(tc.tile_pool(name="const", bufs=1))
        rpool = ctx.enter_context(tc.tile_pool(name="rsb", bufs=1))
        xpool = ctx.enter_context(tc.tile_pool(name="xin", bufs=10))
        hpool = ctx.enter_context(tc.tile_pool(name="hin", bufs=1))
        opool = ctx.enter_context(tc.tile_pool(name="osb", bufs=8))
        ipsum = ctx.enter_context(tc.tile_pool(name="ips", bufs=4, space="PSUM"))

        # constants
        bsb = const_pool.tile([128, 10, 128], DT)
        nc.sync.dma_start(out=bsb[:], in_=bandsD[:])
        bias_sb = const_pool.tile([128, 1], F32)
        nc.sync.dma_start(out=bias_sb[:], in_=biasD[:])

        def conv_mms(psum, xt, ci, K, M, first, last):
            """3 dx matmuls per column chunk for conv ci on tile xt."""
            for (ca, cb) in chunks:
                for dxi in (1, 0, 2):
                    nc.tensor.matmul(
                        psum[0:M, ca:cb],
                        bsb[0:K, ci * 3 + dxi, 0:M],
                        xt[0:K, ca + dxi:cb + dxi],
                        start=(first and dxi == 1),
                        stop=(last and dxi == 2),
                    )

        def residual(h_src, c_src, bg_src, K, M, tag):
            """R band = conv(H0,W9) + conv(C0,W10) + bias + bgate3."""
            ht = hpool.tile([128, Wp], DT, tag="ht" + tag)
            ct = hpool.tile([128, Wp], DT, tag="ct" + tag)
            bgt = hpool.tile([128, W], F32, tag="bgt" + tag)
            nc.sync.dma_start(out=ht[0:K, :], in_=h_src)
            nc.sync.dma_start(out=ct[0:K, :], in_=c_src)
            nc.sync.dma_start(out=bgt[0:M, :], in_=bg_src)
            psum = ipsum.tile([128, W], F32, tag="ips")
            conv_mms(psum, ht, 1, K, M, True, False)
            conv_mms(psum, ct, 2, K, M, False, True)
            R = rpool.tile([128, W], DT, tag="R" + tag)
            nc.vector.scalar_tensor_tensor(
                out=R[0:M, :], in0=psum[0:M, :], scalar=bias_sb[0:M, :],
                in1=bgt[0:M, :],
                op0=mybir.AluOpType.add, op1=mybir.AluOpType.add,
            )
            return R

        def image_band(x_src, out_dst, R, K, M):
            xt = xpool.tile([128, Wp], DT, tag="xt")
            nc.sync.dma_start(out=xt[0:K, :], in_=x_src)
            psum = ipsum.tile([128, W], F32, tag="ips")
            conv_mms(psum, xt, 0, K, M, True, False)
            # add R through an identity-lhsT matmul, completing the group
            for ki, (ca, cb) in enumerate(chunks):
                nc.tensor.matmul(
                    psum[0:M, ca:cb],
                    bsb[0:M, 9, 0:M],
                    R[0:M, ca:cb],
                    start=False, stop=(ki == len(chunks) - 1),
                )
            ot = opool.tile([128, W], F32, tag="ot")
            nc.scalar.activation(
                ot[0:M, :], psum[0:M, :], mybir.ActivationFunctionType.Sigmoid,
            )
            nc.sync.dma_start(out=out_dst, in_=ot[0:M, :])

        # ---- main band: R, then all B images ----
        Rm = residual(h0m[:], c0m[:], bg3m[:], 128, TR, "m")
        # ---- tail residual up front too (loads are tiny) ----
        Rt = residual(h0t[:], c0t[:], bg3t[:], KT, TT, "t")

        for img in range(B):
            image_band(xm[img], outm[img], Rm, 128, TR)
            if img % (B // BH) == 0:  # interleave a home-image tail every 8th
                g = img // (B // BH)
                if g < BH:
                    image_band(xt4[g], outt[g], Rt, KT, TT)

    nc.compile()
    return nc


_NC_CACHE = {}


def _get_nc(B, H, W, dt_mode="fp16", n_cores=N_CORES):
    key = (B, H, W, dt_mode, n_cores)
    if key not in _NC_CACHE:
        _NC_CACHE[key] = _build_nc(B, H, W, dt_mode, n_cores)
    return _NC_CACHE[key]


def _make_inmaps(x, H0, C0, Wconv, bconv, bgate, n_cores, dt_mode="fp16"):
    B = x.shape[0]
    H, W = x.shape[2], x.shape[3]
    BH = B // n_cores
    TT = H - TR * n_cores
    KT = TT + 2

    ndt = np.float16 if dt_mode == "fp16" else np.float32
    xp = np.pad(np.asarray(x, ndt).reshape(B, H, W), ((0, 0), (1, 1), (1, 1)))
    h0p = np.pad(np.asarray(H0, ndt)[0, 0], 1)
    c0p = np.pad(np.asarray(C0, ndt)[0, 0], 1)
    bg3 = np.ascontiguousarray(np.asarray(bgate, np.float32)[3])
    Wc = np.asarray(Wconv, np.float32)
    bands = _build_bands([Wc[8, 0, 0], Wc[9, 0, 0], Wc[10, 0, 0]]).astype(ndt)
    bc = np.asarray(bconv, np.float32)
    bias = np.full((128, 1), bc[8] + bc[9] + bc[10], np.float32)

    maps = []
    for c in range(n_cores):
        r0 = TR * c
        maps.append({
            "xm": np.ascontiguousarray(xp[:, r0:r0 + 128, :]),
            "h0m": np.ascontiguousarray(h0p[r0:r0 + 128, :]),
            "c0m": np.ascontiguousarray(c0p[r0:r0 + 128, :]),
            "bg3m": np.ascontiguousarray(bg3[r0:r0 + TR, :]),
            "xt4": np.ascontiguousarray(xp[c * BH:(c + 1) * BH, H - TT:H + 2, :]),
            "h0t": np.ascontiguousarray(h0p[H - TT:H + 2, :]),
            "c0t": np.ascontiguousarray(c0p[H - TT:H + 2, :]),
            "bg3t": np.ascontiguousarray(bg3[H - TT:, :]),
            "bands": bands, "biasv": bias,
        })
    return maps


def kernel(x, H0, C0, Wconv, bconv, bgate):
    B, _, H, W = x.shape
    dt_mode = os.environ.get("CONV_DT", "fp16")
    nc = _get_nc(B, H, W, dt_mode)
    in_maps = _make_inmaps(x, H0, C0, Wconv, bconv, bgate, N_CORES, dt_mode)
    trace = os.environ.get("CONV_TRACE", "") == "1"
    res = run_bass_kernel_spmd(nc, in_maps, list(range(N_CORES)), trace=trace)
    if trace:
        kernel.last_exec_time_ns = res.exec_time_ns
        kernel.last_results = res

    out = np.empty((B, H, W), np.float32)
    BH = B // N_CORES
    TT = H - TR * N_CORES
    for c in range(N_CORES):
        r0 = TR * c
        out[:, r0:r0 + TR, :] = res.results[c]["outm"]
        if TT:
            out[c * BH:(c + 1) * BH, H - TT:, :] = res.results[c]["outt"]
    return out.reshape(B, 1, H, W)


# revision 12
# speedup vs baseline: 2.8867x; 1.0758x over previous
"""ConvLSTM forward (ot gate only) as a Trainium2 Bass kernel.

The reference module returns only
    ot = sigmoid(conv(x, W8)+b8 + conv(H0, W9)+b9 + conv(C0, W10)+b10 + bgate[3])
(the it/ft/Ct computations are dead code).  The H0/C0 terms are
batch-independent, so each core computes a residual band
    R = conv(H0, W9) + conv(C0, W10) + bgate[3] + (b8+b9+b10)
once and then per image computes  sigmoid(conv(x_i, W8) + R).

Sharding: spatial over H.  Core c produces output rows [126c, 126c+126)
of all 32 images (x slabs carry a 1-row halo), so H0/C0/bgate loads and
the R conv are sharded 8-ways instead of replicated.  The 16-row
remainder (1024 = 8*126 + 16) is batch-sharded: each core computes the
tail rows of its 4 "home" images.

The 3x3 'same' conv runs on the TensorEngine.  Inputs are zero-padded
by one row/column on each side (host-side), so a band of 126 output
rows reads a 128-row input tile and every core uses the same
tridiagonal 128x128 "band" lhsT matrices:
    psum[m, c] += sum_k Band_dx[k, m] * x[k, c+dx]
accumulates the full 3x3 conv over three matmuls per 512-column chunk
(horizontal taps are shifted column APs of the same SBUF tile; matmuls
run as float32r for full PE rate).  The residual add runs on the Vector
engine (PSUM + R -> SBUF) and the sigmoid on the Scalar engine.
"""

import os
from contextlib import ExitStack

import numpy as np

import concourse.bass as bass
import concourse.bacc as bacc
import concourse.mybir as mybir
from concourse import tile
from concourse.bass_utils import run_bass_kernel_spmd

F32 = mybir.dt.float32
F32R = mybir.dt.float32r
F16 = mybir.dt.float16
_DT_MAP = {"f32r": F32R, "fp16": F16, "f32": F32}

N_CORES = 8
TR = 126  # output rows per band (input tiles carry a 1-row halo each side)
CH = 512  # psum column chunk (max fp32 moving free dim)


def _build_bands(w_list):
    """Tridiagonal lhsT matrices for the vertical conv taps.

    Input tile partition k holds padded image row r0+k (= image row
    r0+k-1); psum partition m holds output image row r0+m.  The tap at
    vertical offset dy reads input partition k = m+dy+1 = m+dyi, so
    Band[m+dyi, m] = w[dyi, dxi].
    Returns [9, 128, 128] for conv ci in (0,1,2) x dxi in (0,1,2),
    pre-transposed to [128, 9, 128] (partition-major) for a linear DMA.
    """
    bands = np.zeros((9, 128, 128), np.float32)
    for ci, w in enumerate(w_list):
        for dxi in range(3):
            B = bands[ci * 3 + dxi]
            for m in range(128):
                for dyi in range(3):
                    k = m + dyi
                    if k < 128:
                        B[k, m] = w[dyi, dxi]
    out = np.zeros((10, 128, 128), np.float32)
    out[:9] = bands
    out[9] = np.eye(128, dtype=np.float32)
    return np.ascontiguousarray(out.transpose(1, 0, 2))  # [128, 10, 128]


def _build_nc(B, H, W, dt_mode="fp16", n_cores=N_CORES):
    """Per-core Bass program (SPMD: same program, different data).

    B: total images (each core sees all of them for its main band).
    Main band: TR output rows; tail: TT = H - 7*TR... computed from H.
    """
    DT = _DT_MAP[dt_mode]
    TT = H - TR * n_cores  # tail rows (batch-sharded), 16 for H=1024
    BH = B // n_cores  # home images per core
    Wp = W + 2
    nc = bacc.Bacc(None, target_bir_lowering=False, debug=False)

    # Main-band inputs: padded rows [126c, 126c+128) of every image.
    xm = nc.dram_tensor("xm", [B, 128, Wp], DT, kind="ExternalInput")
    h0m = nc.dram_tensor("h0m", [128, Wp], DT, kind="ExternalInput")
    c0m = nc.dram_tensor("c0m", [128, Wp], DT, kind="ExternalInput")
    bg3m = nc.dram_tensor("bg3m", [TR, W], F32, kind="ExternalInput")
    # Tail inputs: padded rows [H-TT, H+2) of the BH home images.
    KT = TT + 2
    xt4 = nc.dram_tensor("xt4", [BH, KT, Wp], DT, kind="ExternalInput")
    h0t = nc.dram_tensor("h0t", [KT, Wp], DT, kind="ExternalInput")
    c0t = nc.dram_tensor("c0t", [KT, Wp], DT, kind="ExternalInput")
    bg3t = nc.dram_tensor("bg3t", [TT, W], F32, kind="ExternalInput")

    bandsD = nc.dram_tensor("bands", [128, 10, 128], DT, kind="ExternalInput")
    biasD = nc.dram_tensor("biasv", [128, 1], F32, kind="ExternalInput")

    outm = nc.dram_tensor("outm", [B, TR, W], F32, kind="ExternalOutput")
    outt = nc.dram_tensor("outt", [BH, TT, W], F32, kind="ExternalOutput")

    chunks = []
    c = 0
    while c < W:
        chunks.append((c, min(c + CH, W)))
        c += CH

    with tile.TileContext(nc) as tc, ExitStack() as ctx:
        const_pool = ctx.enter_context(tc.tile_pool(name="const", bufs=1))
        rpool = ctx.enter_context(tc.tile_pool(name="rsb", bufs=1))
        xpool = ctx.enter_context(tc.tile_pool(name="xin", bufs=14))
        hpool = ctx.enter_context(tc.tile_pool(name="hin", bufs=1))
        opool = ctx.enter_context(tc.tile_pool(name="osb", bufs=12))
        ipsum = ctx.enter_context(tc.tile_pool(name="ips", bufs=4, space="PSUM"))

        # constants
        bsb = const_pool.tile([128, 10, 128], DT)
        nc.sync.dma_start(out=bsb[:], in_=bandsD[:])
        bias_sb = const_pool.tile([128, 1], F32)
        nc.sync.dma_start(out=bias_sb[:], in_=biasD[:])

        def conv_mms(psum, xt, ci, K, M, first, last):
            """3 dx matmuls per column chunk for conv ci on tile xt."""
            for (ca, cb) in chunks:
                for dxi in (1, 0, 2):
                    nc.tensor.matmul(
                        psum[0:M, ca:cb],
                        bsb[0:K, ci * 3 + dxi, 0:M],
                        xt[0:K, ca + dxi:cb + dxi],
                        start=(first and dxi == 1),
                        stop=(last and dxi == 2),
                    )

        def residual(h_src, c_src, bg_src, K, M, tag):
            """R band = conv(H0,W9) + conv(C0,W10) + bias + bgate3."""
            ht = hpool.tile([128, Wp], DT, tag="ht" + tag)
            ct = hpool.tile([128, Wp], DT, tag="ct" + tag)
            bgt = hpool.tile([128, W], F32, tag="bgt" + tag)
            nc.sync.dma_start(out=ht[0:K, :], in_=h_src)
            nc.sync.dma_start(out=ct[0:K, :], in_=c_src)
            nc.sync.dma_start(out=bgt[0:M, :], in_=bg_src)
            psum = ipsum.tile([128, W], F32, tag="ips")
            conv_mms(psum, ht, 1, K, M, True, False)
            conv_mms(psum, ct, 2, K, M, False, True)
            R = rpool.tile([128, W], DT, tag="R" + tag)
            nc.vector.scalar_tensor_tensor(
                out=R[0:M, :], in0=psum[0:M, :], scalar=bias_sb[0:M, :],
                in1=bgt[0:M, :],
                op0=mybir.AluOpType.add, op1=mybir.AluOpType.add,
            )
            return R

        def image_band(x_src, out_dst, R, K, M):
            xt = xpool.tile([128, Wp], DT, tag="xt")
            nc.sync.dma_start(out=xt[0:K, :], in_=x_src)
            psum = ipsum.tile([128, W], F32, tag="ips")
            conv_mms(psum, xt, 0, K, M, True, False)
            # add R through an identity-lhsT matmul, completing the group
            for ki, (ca, cb) in enumerate(chunks):
                nc.tensor.matmul(
                    psum[0:M, ca:cb],
                    bsb[0:M, 9, 0:M],
                    R[0:M, ca:cb],
                    start=False, stop=(ki == len(chunks) - 1),
                )
            ot = opool.tile([128, W], F32, tag="ot")
            nc.scalar.activation(
                ot[0:M, :], psum[0:M, :], mybir.ActivationFunctionType.Sigmoid,
            )
            nc.gpsimd.dma_start(out=out_dst, in_=ot[0:M, :])

        # ---- main band: R, then all B images ----
        Rm = residual(h0m[:], c0m[:], bg3m[:], 128, TR, "m")
        # ---- tail residual up front too (loads are tiny) ----
        Rt = residual(h0t[:], c0t[:], bg3t[:], KT, TT, "t")

        for img in range(B):
            image_band(xm[img], outm[img], Rm, 128, TR)
            if img % (B // BH) == 0:  # interleave a home-image tail every 8th
                g = img // (B // BH)
                if g < BH:
                    image_band(xt4[g], outt[g], Rt, KT, TT)

    nc.compile()
    return nc


_NC_CACHE = {}


def _get_nc(B, H, W, dt_mode="fp16", n_cores=N_CORES):
    key = (B, H, W, dt_mode, n_cores)
    if key not in _NC_CACHE:
        _NC_CACHE[key] = _build_nc(B, H, W, dt_mode, n_cores)
    return _NC_CACHE[key]


def _make_inmaps(x, H0, C0, Wconv, bconv, bgate, n_cores, dt_mode="fp16"):
    B = x.shape[0]
    H, W = x.shape[2], x.shape[3]
    BH = B // n_cores
    TT = H - TR * n_cores
    KT = TT + 2

    ndt = np.float16 if dt_mode == "fp16" else np.float32
    xp = np.pad(np.asarray(x, ndt).reshape(B, H, W), ((0, 0), (1, 1), (1, 1)))
    h0p = np.pad(np.asarray(H0, ndt)[0, 0], 1)
    c0p = np.pad(np.asarray(C0, ndt)[0, 0], 1)
    bg3 = np.ascontiguousarray(np.asarray(bgate, np.float32)[3])
    Wc = np.asarray(Wconv, np.float32)
    bands = _build_bands([Wc[8, 0, 0], Wc[9, 0, 0], Wc[10, 0, 0]]).astype(ndt)
    bc = np.asarray(bconv, np.float32)
    bias = np.full((128, 1), bc[8] + bc[9] + bc[10], np.float32)

    maps = []
    for c in range(n_cores):
        r0 = TR * c
        maps.append({
            "xm": np.ascontiguousarray(xp[:, r0:r0 + 128, :]),
            "h0m": np.ascontiguousarray(h0p[r0:r0 + 128, :]),
            "c0m": np.ascontiguousarray(c0p[r0:r0 + 128, :]),
            "bg3m": np.ascontiguousarray(bg3[r0:r0 + TR, :]),
            "xt4": np.ascontiguousarray(xp[c * BH:(c + 1) * BH, H - TT:H + 2, :]),
            "h0t": np.ascontiguousarray(h0p[H - TT:H + 2, :]),
            "c0t": np.ascontiguousarray(c0p[H - TT:H + 2, :]),
            "bg3t": np.ascontiguousarray(bg3[H - TT:, :]),
            "bands": bands, "biasv": bias,
        })
    return maps


def kernel(x, H0, C0, Wconv, bconv, bgate):
    B, _, H, W = x.shape
    dt_mode = os.environ.get("CONV_DT", "fp16")
    nc = _get_nc(B, H, W, dt_mode)
    in_maps = _make_inmaps(x, H0, C0, Wconv, bconv, bgate, N_CORES, dt_mode)
    trace = os.environ.get("CONV_TRACE", "") == "1"
    res = run_bass_kernel_spmd(nc, in_maps, list(range(N_CORES)), trace=trace)
    if trace:
        kernel.last_exec_time_ns = res.exec_time_ns
        kernel.last_results = res

    out = np.empty((B, H, W), np.float32)
    BH = B // N_CORES
    TT = H - TR * N_CORES
    for c in range(N_CORES):
        r0 = TR * c
        out[:, r0:r0 + TR, :] = res.results[c]["outm"]
        if TT:
            out[c * BH:(c + 1) * BH, H - TT:, :] = res.results[c]["outt"]
    return out.reshape(B, 1, H, W)


# revision 13
# speedup vs baseline: 2.9976x; 1.0384x over previous
"""ConvLSTM forward (ot gate only) as a Trainium2 Bass kernel.

The reference module returns only
    ot = sigmoid(conv(x, W8)+b8 + conv(H0, W9)+b9 + conv(C0, W10)+b10 + bgate[3])
(the it/ft/Ct computations are dead code).  The H0/C0 terms are
batch-independent, so each core computes a residual band
    R = conv(H0, W9) + conv(C0, W10) + bgate[3] + (b8+b9+b10)
once and then per image computes  sigmoid(conv(x_i, W8) + R).

Sharding: spatial over H.  Core c produces output rows [126c, 126c+126)
of all 32 images (x slabs carry a 1-row halo), so H0/C0/bgate loads and
the R conv are sharded 8-ways instead of replicated.  The 16-row
remainder (1024 = 8*126 + 16) is batch-sharded: each core computes the
tail rows of its 4 "home" images.

The 3x3 'same' conv runs on the TensorEngine.  Inputs are zero-padded
by one row/column on each side (host-side), so a band of 126 output
rows reads a 128-row input tile and every core uses the same
tridiagonal 128x128 "band" lhsT matrices:
    psum[m, c] += sum_k Band_dx[k, m] * x[k, c+dx]
accumulates the full 3x3 conv over three matmuls per 512-column chunk
(horizontal taps are shifted column APs of the same SBUF tile; matmuls
run as float32r for full PE rate).  The residual add runs on the Vector
engine (PSUM + R -> SBUF) and the sigmoid on the Scalar engine.
"""

import os
from contextlib import ExitStack

import numpy as np

import concourse.bass as bass
import concourse.bacc as bacc
import concourse.mybir as mybir
from concourse import tile
from concourse.bass_utils import run_bass_kernel_spmd

F32 = mybir.dt.float32
F32R = mybir.dt.float32r
F16 = mybir.dt.float16
_DT_MAP = {"f32r": F32R, "fp16": F16, "f32": F32}

N_CORES = 8
TR = 126  # output rows per band (input tiles carry a 1-row halo each side)
CH = 512  # psum column chunk (max fp32 moving free dim)


def _build_bands(w_list):
    """Tridiagonal lhsT matrices for the vertical conv taps.

    Input tile partition k holds padded image row r0+k (= image row
    r0+k-1); psum partition m holds output image row r0+m.  The tap at
    vertical offset dy reads input partition k = m+dy+1 = m+dyi, so
    Band[m+dyi, m] = w[dyi, dxi].
    Returns [9, 128, 128] for conv ci in (0,1,2) x dxi in (0,1,2),
    pre-transposed to [128, 9, 128] (partition-major) for a linear DMA.
    """
    bands = np.zeros((9, 128, 128), np.float32)
    for ci, w in enumerate(w_list):
        for dxi in range(3):
            B = bands[ci * 3 + dxi]
            for m in range(128):
                for dyi in range(3):
                    k = m + dyi
                    if k < 128:
                        B[k, m] = w[dyi, dxi]
    out = np.zeros((10, 128, 128), np.float32)
    out[:9] = bands
    out[9] = np.eye(128, dtype=np.float32)
    return np.ascontiguousarray(out.transpose(1, 0, 2))  # [128, 10, 128]


def _build_nc(B, H, W, dt_mode="fp16", n_cores=N_CORES):
    """Per-core Bass program (SPMD: same program, different data).

    B: total images (each core sees all of them for its main band).
    Main band: TR output rows; tail: TT = H - 7*TR... computed from H.
    """
    DT = _DT_MAP[dt_mode]
    TT = H - TR * n_cores  # tail rows (batch-sharded), 16 for H=1024
    BH = B // n_cores  # home images per core
    Wp = W + 2
    nc = bacc.Bacc(None, target_bir_lowering=False, debug=False)

    # Main-band inputs: padded rows [126c, 126c+128) of every image.
    xm = nc.dram_tensor("xm", [B, 128, Wp], DT, kind="ExternalInput")
    h0m = nc.dram_tensor("h0m", [128, Wp], DT, kind="ExternalInput")
    c0m = nc.dram_tensor("c0m", [128, Wp], DT, kind="ExternalInput")
    bg3m = nc.dram_tensor("bg3m", [TR, W], F32, kind="ExternalInput")
    # Tail inputs: padded rows [H-TT, H+2) of the BH home images.
    KT = TT + 2
    xt4 = nc.dram_tensor("xt4", [BH, KT, Wp], DT, kind="ExternalInput")
    h0t = nc.dram_tensor("h0t", [KT, Wp], DT, kind="ExternalInput")
    c0t = nc.dram_tensor("c0t", [KT, Wp], DT, kind="ExternalInput")
    bg3t = nc.dram_tensor("bg3t", [TT, W], F32, kind="ExternalInput")

    bandsD = nc.dram_tensor("bands", [128, 10, 128], DT, kind="ExternalInput")
    biasD = nc.dram_tensor("biasv", [128, 1], F32, kind="ExternalInput")

    outm = nc.dram_tensor("outm", [B, TR, W], F32, kind="ExternalOutput")
    outt = nc.dram_tensor("outt", [BH, TT, W], F32, kind="ExternalOutput")

    chunks = []
    c = 0
    while c < W:
        chunks.append((c, min(c + CH, W)))
        c += CH

    with tile.TileContext(nc) as tc, ExitStack() as ctx:
        const_pool = ctx.enter_context(tc.tile_pool(name="const", bufs=1))
        rpool = ctx.enter_context(tc.tile_pool(name="rsb", bufs=1))
        xpool = ctx.enter_context(tc.tile_pool(name="xin", bufs=14))
        hpool = ctx.enter_context(tc.tile_pool(name="hin", bufs=1))
        opool = ctx.enter_context(tc.tile_pool(name="osb", bufs=12))
        ipsum = ctx.enter_context(tc.tile_pool(name="ips", bufs=4, space="PSUM"))

        # constants
        bsb = const_pool.tile([128, 10, 128], DT)
        nc.sync.dma_start(out=bsb[:], in_=bandsD[:])
        bias_sb = const_pool.tile([128, 1], F32)
        nc.sync.dma_start(out=bias_sb[:], in_=biasD[:])

        def conv_mms(psum, xt, ci, K, M, first, last):
            """3 dx matmuls per column chunk for conv ci on tile xt."""
            for (ca, cb) in chunks:
                for dxi in (1, 0, 2):
                    nc.tensor.matmul(
                        psum[0:M, ca:cb],
                        bsb[0:K, ci * 3 + dxi, 0:M],
                        xt[0:K, ca + dxi:cb + dxi],
                        start=(first and dxi == 1),
                        stop=(last and dxi == 2),
                    )

        def residual(h_src, c_src, bg_src, K, M, tag):
            """R band = conv(H0,W9) + conv(C0,W10) + bias + bgate3."""
            ht = hpool.tile([128, Wp], DT, tag="ht" + tag)
            ct = hpool.tile([128, Wp], DT, tag="ct" + tag)
            bgt = hpool.tile([128, W], F32, tag="bgt" + tag)
            nc.sync.dma_start(out=ht[0:K, :], in_=h_src)
            nc.sync.dma_start(out=ct[0:K, :], in_=c_src)
            nc.sync.dma_start(out=bgt[0:M, :], in_=bg_src)
            psum = ipsum.tile([128, W], F32, tag="ips")
            conv_mms(psum, ht, 1, K, M, True, False)
            conv_mms(psum, ct, 2, K, M, False, True)
            R = rpool.tile([128, W], DT, tag="R" + tag)
            nc.vector.scalar_tensor_tensor(
                out=R[0:M, :], in0=psum[0:M, :], scalar=bias_sb[0:M, :],
                in1=bgt[0:M, :],
                op0=mybir.AluOpType.add, op1=mybir.AluOpType.add,
            )
            return R

        def image_band(x_src, out_dst, R, K, M):
            xt = xpool.tile([128, Wp], DT, tag="xt")
            nc.sync.dma_start(out=xt[0:K, :], in_=x_src)
            psum = ipsum.tile([128, W], F32, tag="ips")
            conv_mms(psum, xt, 0, K, M, True, True)
            ot = opool.tile([128, W], F32, tag="ot")
            nc.vector.tensor_add(out=ot[0:M, :], in0=psum[0:M, :], in1=R[0:M, :])
            nc.scalar.activation(
                ot[0:M, :], ot[0:M, :], mybir.ActivationFunctionType.Sigmoid,
            )
            nc.gpsimd.dma_start(out=out_dst, in_=ot[0:M, :])

        # ---- main band: R, then all B images ----
        Rm = residual(h0m[:], c0m[:], bg3m[:], 128, TR, "m")
        # ---- tail residual up front too (loads are tiny) ----
        Rt = residual(h0t[:], c0t[:], bg3t[:], KT, TT, "t")

        for img in range(B):
            image_band(xm[img], outm[img], Rm, 128, TR)
            if img % (B // BH) == 0:  # interleave a home-image tail every 8th
                g = img // (B // BH)
                if g < BH:
                    image_band(xt4[g], outt[g], Rt, KT, TT)

    nc.compile()
    return nc


_NC_CACHE = {}


def _get_nc(B, H, W, dt_mode="fp16", n_cores=N_CORES):
    key = (B, H, W, dt_mode, n_cores)
    if key not in _NC_CACHE:
        _NC_CACHE[key] = _build_nc(B, H, W, dt_mode, n_cores)
    return _NC_CACHE[key]


def _make_inmaps(x, H0, C0, Wconv, bconv, bgate, n_cores, dt_mode="fp16"):
    B = x.shape[0]
    H, W = x.shape[2], x.shape[3]
    BH = B // n_cores
    TT = H - TR * n_cores
    KT = TT + 2

    ndt = np.float16 if dt_mode == "fp16" else np.float32
    xp = np.pad(np.asarray(x, ndt).reshape(B, H, W), ((0, 0), (1, 1), (1, 1)))
    h0p = np.pad(np.asarray(H0, ndt)[0, 0], 1)
    c0p = np.pad(np.asarray(C0, ndt)[0, 0], 1)
    bg3 = np.ascontiguousarray(np.asarray(bgate, np.float32)[3])
    Wc = np.asarray(Wconv, np.float32)
    bands = _build_bands([Wc[8, 0, 0], Wc[9, 0, 0], Wc[10, 0, 0]]).astype(ndt)
    bc = np.asarray(bconv, np.float32)
    bias = np.full((128, 1), bc[8] + bc[9] + bc[10], np.float32)

    maps = []
    for c in range(n_cores):
        r0 = TR * c
        maps.append({
            "xm": np.ascontiguousarray(xp[:, r0:r0 + 128, :]),
            "h0m": np.ascontiguousarray(h0p[r0:r0 + 128, :]),
            "c0m": np.ascontiguousarray(c0p[r0:r0 + 128, :]),
            "bg3m": np.ascontiguousarray(bg3[r0:r0 + TR, :]),
            "xt4": np.ascontiguousarray(xp[c * BH:(c + 1) * BH, H - TT:H + 2, :]),
            "h0t": np.ascontiguousarray(h0p[H - TT:H + 2, :]),
            "c0t": np.ascontiguousarray(c0p[H - TT:H + 2, :]),
            "bg3t": np.ascontiguousarray(bg3[H - TT:, :]),
            "bands": bands, "biasv": bias,
        })
    return maps


def kernel(x, H0, C0, Wconv, bconv, bgate):
    B, _, H, W = x.shape
    dt_mode = os.environ.get("CONV_DT", "fp16")
    nc = _get_nc(B, H, W, dt_mode)
    in_maps = _make_inmaps(x, H0, C0, Wconv, bconv, bgate, N_CORES, dt_mode)
    trace = os.environ.get("CONV_TRACE", "") == "1"
    res = run_bass_kernel_spmd(nc, in_maps, list(range(N_CORES)), trace=trace)
    if trace:
        kernel.last_exec_time_ns = res.exec_time_ns
        kernel.last_results = res

    out = np.empty((B, H, W), np.float32)
    BH = B // N_CORES
    TT = H - TR * N_CORES
    for c in range(N_CORES):
        r0 = TR * c
        out[:, r0:r0 + TR, :] = res.results[c]["outm"]
        if TT:
            out[c * BH:(c + 1) * BH, H - TT:, :] = res.results[c]["outt"]
    return out.reshape(B, 1, H, W)


# revision 14
# speedup vs baseline: 3.9172x; 1.3068x over previous
"""ConvLSTM forward (ot gate only) as a Trainium2 Bass kernel.

The reference module returns only
    ot = sigmoid(conv(x, W8)+b8 + conv(H0, W9)+b9 + conv(C0, W10)+b10 + bgate[3])
(the it/ft/Ct computations are dead code).  The H0/C0 terms are
batch-independent, so each core computes a residual band
    R = conv(H0, W9) + conv(C0, W10) + bgate[3] + (b8+b9+b10)
once and then per image computes  sigmoid(conv(x_i, W8) + R).

Sharding: spatial over H.  Core c produces output rows [126c, 126c+126)
of all 32 images (x slabs carry a 1-row halo), so H0/C0/bgate loads and
the R conv are sharded 8-ways instead of replicated.  The 16-row
remainder (1024 = 8*126 + 16) is batch-sharded: each core computes the
tail rows of its 4 "home" images.

The 3x3 'same' conv runs on the TensorEngine.  Inputs are zero-padded
by one row/column on each side (host-side), so a band of 126 output
rows reads a 128-row input tile and every core uses the same
tridiagonal 128x128 "band" lhsT matrices:
    psum[m, c] += sum_k Band_dx[k, m] * x[k, c+dx]
accumulates the full 3x3 conv over three matmuls per 512-column chunk
(horizontal taps are shifted column APs of the same SBUF tile; matmuls
run as float32r for full PE rate).  The residual add runs on the Vector
engine (PSUM + R -> SBUF) and the sigmoid on the Scalar engine.
"""

import os
from contextlib import ExitStack

import numpy as np

import concourse.bass as bass
import concourse.bacc as bacc
import concourse.mybir as mybir
from concourse import tile
from concourse.bass_utils import run_bass_kernel_spmd

F32 = mybir.dt.float32
F32R = mybir.dt.float32r
F16 = mybir.dt.float16
_DT_MAP = {"f32r": F32R, "fp16": F16, "f32": F32}

N_CORES = 8
TR = 126  # output rows per band (input tiles carry a 1-row halo each side)
CH = 512  # psum column chunk (max fp32 moving free dim)


def _build_bands(w_list):
    """Tridiagonal lhsT matrices for the vertical conv taps.

    Input tile partition k holds padded image row r0+k (= image row
    r0+k-1); psum partition m holds output image row r0+m.  The tap at
    vertical offset dy reads input partition k = m+dy+1 = m+dyi, so
    Band[m+dyi, m] = w[dyi, dxi].
    Returns [9, 128, 128] for conv ci in (0,1,2) x dxi in (0,1,2),
    pre-transposed to [128, 9, 128] (partition-major) for a linear DMA.
    """
    bands = np.zeros((9, 128, 128), np.float32)
    for ci, w in enumerate(w_list):
        for dxi in range(3):
            B = bands[ci * 3 + dxi]
            for m in range(128):
                for dyi in range(3):
                    k = m + dyi
                    if k < 128:
                        B[k, m] = w[dyi, dxi]
    out = np.zeros((10, 128, 128), np.float32)
    out[:9] = bands
    out[9] = np.eye(128, dtype=np.float32)
    return np.ascontiguousarray(out.transpose(1, 0, 2))  # [128, 10, 128]


def _build_nc(B, H, W, dt_mode="fp16", out16=False, n_cores=N_CORES):
    """Per-core Bass program (SPMD: same program, different data).

    B: total images (each core sees all of them for its main band).
    Main band: TR output rows; tail: TT = H - 7*TR... computed from H.
    """
    DT = _DT_MAP[dt_mode]
    TT = H - TR * n_cores  # tail rows (batch-sharded), 16 for H=1024
    BH = B // n_cores  # home images per core
    Wp = W + 2
    nc = bacc.Bacc(None, target_bir_lowering=False, debug=False)

    # Main-band inputs: padded rows [126c, 126c+128) of every image.
    xm = nc.dram_tensor("xm", [B, 128, Wp], DT, kind="ExternalInput")
    h0m = nc.dram_tensor("h0m", [128, Wp], DT, kind="ExternalInput")
    c0m = nc.dram_tensor("c0m", [128, Wp], DT, kind="ExternalInput")
    bg3m = nc.dram_tensor("bg3m", [TR, W], F32, kind="ExternalInput")
    # Tail inputs: padded rows [H-TT, H+2) of the BH home images.
    KT = TT + 2
    xt4 = nc.dram_tensor("xt4", [BH, KT, Wp], DT, kind="ExternalInput")
    h0t = nc.dram_tensor("h0t", [KT, Wp], DT, kind="ExternalInput")
    c0t = nc.dram_tensor("c0t", [KT, Wp], DT, kind="ExternalInput")
    bg3t = nc.dram_tensor("bg3t", [TT, W], F32, kind="ExternalInput")

    bandsD = nc.dram_tensor("bands", [128, 10, 128], DT, kind="ExternalInput")
    biasD = nc.dram_tensor("biasv", [128, 1], F32, kind="ExternalInput")

    ODT = F16 if out16 else F32
    outm = nc.dram_tensor("outm", [B, TR, W], ODT, kind="ExternalOutput")
    outt = nc.dram_tensor("outt", [BH, TT, W], ODT, kind="ExternalOutput")

    chunks = []
    c = 0
    while c < W:
        chunks.append((c, min(c + CH, W)))
        c += CH

    with tile.TileContext(nc) as tc, ExitStack() as ctx:
        const_pool = ctx.enter_context(tc.tile_pool(name="const", bufs=1))
        rpool = ctx.enter_context(tc.tile_pool(name="rsb", bufs=1))
        xpool = ctx.enter_context(tc.tile_pool(name="xin", bufs=14))
        hpool = ctx.enter_context(tc.tile_pool(name="hin", bufs=1))
        opool = ctx.enter_context(tc.tile_pool(name="osb", bufs=12))
        ipsum = ctx.enter_context(tc.tile_pool(name="ips", bufs=4, space="PSUM"))

        # constants
        bsb = const_pool.tile([128, 10, 128], DT)
        nc.sync.dma_start(out=bsb[:], in_=bandsD[:])
        bias_sb = const_pool.tile([128, 1], F32)
        nc.sync.dma_start(out=bias_sb[:], in_=biasD[:])

        def conv_mms(psum, xt, ci, K, M, first, last):
            """3 dx matmuls per column chunk for conv ci on tile xt."""
            for (ca, cb) in chunks:
                for dxi in (1, 0, 2):
                    nc.tensor.matmul(
                        psum[0:M, ca:cb],
                        bsb[0:K, ci * 3 + dxi, 0:M],
                        xt[0:K, ca + dxi:cb + dxi],
                        start=(first and dxi == 1),
                        stop=(last and dxi == 2),
                    )

        def residual(h_src, c_src, bg_src, K, M, tag):
            """R band = conv(H0,W9) + conv(C0,W10) + bias + bgate3."""
            ht = hpool.tile([128, Wp], DT, tag="ht" + tag)
            ct = hpool.tile([128, Wp], DT, tag="ct" + tag)
            bgt = hpool.tile([128, W], F32, tag="bgt" + tag)
            nc.sync.dma_start(out=ht[0:K, :], in_=h_src)
            nc.sync.dma_start(out=ct[0:K, :], in_=c_src)
            nc.sync.dma_start(out=bgt[0:M, :], in_=bg_src)
            psum = ipsum.tile([128, W], F32, tag="ips")
            conv_mms(psum, ht, 1, K, M, True, False)
            conv_mms(psum, ct, 2, K, M, False, True)
            R = rpool.tile([128, W], DT, tag="R" + tag)
            nc.vector.scalar_tensor_tensor(
                out=R[0:M, :], in0=psum[0:M, :], scalar=bias_sb[0:M, :],
                in1=bgt[0:M, :],
                op0=mybir.AluOpType.add, op1=mybir.AluOpType.add,
            )
            return R

        def image_band(x_src, out_dst, R, K, M):
            xt = xpool.tile([128, Wp], DT, tag="xt")
            nc.sync.dma_start(out=xt[0:K, :], in_=x_src)
            psum = ipsum.tile([128, W], F32, tag="ips")
            conv_mms(psum, xt, 0, K, M, True, True)
            ot = opool.tile([128, W], F32, tag="ot")
            nc.vector.tensor_add(out=ot[0:M, :], in0=psum[0:M, :], in1=R[0:M, :])
            if out16:
                o16 = opool.tile([128, W], F16, tag="o16")
                nc.scalar.activation(
                    o16[0:M, :], ot[0:M, :], mybir.ActivationFunctionType.Sigmoid,
                )
                nc.gpsimd.dma_start(out=out_dst, in_=o16[0:M, :])
            else:
                nc.scalar.activation(
                    ot[0:M, :], ot[0:M, :], mybir.ActivationFunctionType.Sigmoid,
                )
                nc.gpsimd.dma_start(out=out_dst, in_=ot[0:M, :])

        # ---- main band: R, then all B images ----
        Rm = residual(h0m[:], c0m[:], bg3m[:], 128, TR, "m")
        # ---- tail residual up front too (loads are tiny) ----
        Rt = residual(h0t[:], c0t[:], bg3t[:], KT, TT, "t")

        for img in range(B):
            image_band(xm[img], outm[img], Rm, 128, TR)
            if img % (B // BH) == 0:  # interleave a home-image tail every 8th
                g = img // (B // BH)
                if g < BH:
                    image_band(xt4[g], outt[g], Rt, KT, TT)

    nc.compile()
    return nc


_NC_CACHE = {}


def _get_nc(B, H, W, dt_mode="fp16", out16=False, n_cores=N_CORES):
    key = (B, H, W, dt_mode, out16, n_cores)
    if key not in _NC_CACHE:
        _NC_CACHE[key] = _build_nc(B, H, W, dt_mode, out16, n_cores)
    return _NC_CACHE[key]


def _make_inmaps(x, H0, C0, Wconv, bconv, bgate, n_cores, dt_mode="fp16"):
    B = x.shape[0]
    H, W = x.shape[2], x.shape[3]
    BH = B // n_cores
    TT = H - TR * n_cores
    KT = TT + 2

    ndt = np.float16 if dt_mode == "fp16" else np.float32
    xp = np.pad(np.asarray(x, ndt).reshape(B, H, W), ((0, 0), (1, 1), (1, 1)))
    h0p = np.pad(np.asarray(H0, ndt)[0, 0], 1)
    c0p = np.pad(np.asarray(C0, ndt)[0, 0], 1)
    bg3 = np.ascontiguousarray(np.asarray(bgate, np.float32)[3])
    Wc = np.asarray(Wconv, np.float32)
    bands = _build_bands([Wc[8, 0, 0], Wc[9, 0, 0], Wc[10, 0, 0]]).astype(ndt)
    bc = np.asarray(bconv, np.float32)
    bias = np.full((128, 1), bc[8] + bc[9] + bc[10], np.float32)

    maps = []
    for c in range(n_cores):
        r0 = TR * c
        maps.append({
            "xm": np.ascontiguousarray(xp[:, r0:r0 + 128, :]),
            "h0m": np.ascontiguousarray(h0p[r0:r0 + 128, :]),
            "c0m": np.ascontiguousarray(c0p[r0:r0 + 128, :]),
            "bg3m": np.ascontiguousarray(bg3[r0:r0 + TR, :]),
            "xt4": np.ascontiguousarray(xp[c * BH:(c + 1) * BH, H - TT:H + 2, :]),
            "h0t": np.ascontiguousarray(h0p[H - TT:H + 2, :]),
            "c0t": np.ascontiguousarray(c0p[H - TT:H + 2, :]),
            "bg3t": np.ascontiguousarray(bg3[H - TT:, :]),
            "bands": bands, "biasv": bias,
        })
    return maps


def kernel(x, H0, C0, Wconv, bconv, bgate):
    B, _, H, W = x.shape
    dt_mode = os.environ.get("CONV_DT", "fp16")
    out16 = os.environ.get("CONV_OUT16", "0") == "1"
    nc = _get_nc(B, H, W, dt_mode, out16)
    in_maps = _make_inmaps(x, H0, C0, Wconv, bconv, bgate, N_CORES, dt_mode)
    trace = os.environ.get("CONV_TRACE", "") == "1"
    res = run_bass_kernel_spmd(nc, in_maps, list(range(N_CORES)), trace=trace)
    if trace:
        kernel.last_exec_time_ns = res.exec_time_ns
        kernel.last_results = res

    out = np.empty((B, H, W), np.float32)
    BH = B // N_CORES
    TT = H - TR * N_CORES
    for c in range(N_CORES):
        r0 = TR * c
        out[:, r0:r0 + TR, :] = res.results[c]["outm"].astype(np.float32)
        if TT:
            out[c * BH:(c + 1) * BH, H - TT:, :] = res.results[c]["outt"].astype(np.float32)
    return out.reshape(B, 1, H, W)


# revision 17
# speedup vs baseline: 4.0920x; 1.0446x over previous
"""ConvLSTM forward (ot gate only) as a Trainium2 Bass kernel.

The reference module returns only
    ot = sigmoid(conv(x, W8)+b8 + conv(H0, W9)+b9 + conv(C0, W10)+b10 + bgate[3])
(the it/ft/Ct computations are dead code).  The H0/C0 terms are
batch-independent, so each core computes a residual band
    R = conv(H0, W9) + conv(C0, W10) + bgate[3] + (b8+b9+b10)
once and then per image computes  sigmoid(conv(x_i, W8) + R).

Sharding: spatial over H.  Core c produces output rows [126c, 126c+126)
of all 32 images (x slabs carry a 1-row halo), so H0/C0/bgate loads and
the R conv are sharded 8-ways instead of replicated.  The 16-row
remainder (1024 = 8*126 + 16) is batch-sharded: each core computes the
tail rows of its 4 "home" images.

The 3x3 'same' conv runs on the TensorEngine.  Inputs are zero-padded
by one row/column on each side (host-side), so a band of 126 output
rows reads a 128-row input tile and every core uses the same
tridiagonal 128x128 "band" lhsT matrices:
    psum[m, c] += sum_k Band_dx[k, m] * x[k, c+dx]
accumulates the full 3x3 conv over three matmuls per 512-column chunk
(horizontal taps are shifted column APs of the same SBUF tile; matmuls
run as float32r for full PE rate).  The residual add runs on the Vector
engine (PSUM + R -> SBUF) and the sigmoid on the Scalar engine.
"""

import os
from contextlib import ExitStack

import numpy as np

import concourse.bass as bass
import concourse.bacc as bacc
import concourse.mybir as mybir
from concourse import tile
from concourse.bass_utils import run_bass_kernel_spmd

F32 = mybir.dt.float32
F32R = mybir.dt.float32r
F16 = mybir.dt.float16
_DT_MAP = {"f32r": F32R, "fp16": F16, "f32": F32}

N_CORES = 8
TR = 126  # output rows per band (input tiles carry a 1-row halo each side)
CH = 512  # psum column chunk (max fp32 moving free dim)


def _build_bands(w_list):
    """Tridiagonal lhsT matrices for the vertical conv taps.

    Input tile partition k holds padded image row r0+k (= image row
    r0+k-1); psum partition m holds output image row r0+m.  The tap at
    vertical offset dy reads input partition k = m+dy+1 = m+dyi, so
    Band[m+dyi, m] = w[dyi, dxi].
    Returns [9, 128, 128] for conv ci in (0,1,2) x dxi in (0,1,2),
    pre-transposed to [128, 9, 128] (partition-major) for a linear DMA.
    """
    bands = np.zeros((9, 128, 128), np.float32)
    for ci, w in enumerate(w_list):
        for dxi in range(3):
            B = bands[ci * 3 + dxi]
            for m in range(128):
                for dyi in range(3):
                    k = m + dyi
                    if k < 128:
                        B[k, m] = w[dyi, dxi]
    out = np.zeros((10, 128, 128), np.float32)
    out[:9] = bands
    out[9] = np.eye(128, dtype=np.float32)
    return np.ascontiguousarray(out.transpose(1, 0, 2))  # [128, 10, 128]


def _build_nc(B, H, W, dt_mode="fp16", out16=False, n_cores=N_CORES):
    """Per-core Bass program (SPMD: same program, different data).

    B: total images (each core sees all of them for its main band).
    Main band: TR output rows; tail: TT = H - 7*TR... computed from H.
    """
    DT = _DT_MAP[dt_mode]
    TT = H - TR * n_cores  # tail rows (batch-sharded), 16 for H=1024
    BH = B // n_cores  # home images per core
    Wp = W + 2
    nc = bacc.Bacc(None, target_bir_lowering=False, debug=False)

    # Main-band inputs: padded rows [126c, 126c+128) of every image.
    xm = nc.dram_tensor("xm", [B, 128, Wp], DT, kind="ExternalInput")
    h0m = nc.dram_tensor("h0m", [128, Wp], DT, kind="ExternalInput")
    c0m = nc.dram_tensor("c0m", [128, Wp], DT, kind="ExternalInput")
    bg3m = nc.dram_tensor("bg3m", [TR, W], F32, kind="ExternalInput")
    # Tail inputs: padded rows [H-TT, H+2) of the BH home images.
    KT = TT + 2
    xt4 = nc.dram_tensor("xt4", [BH, KT, Wp], DT, kind="ExternalInput")
    h0t = nc.dram_tensor("h0t", [KT, Wp], DT, kind="ExternalInput")
    c0t = nc.dram_tensor("c0t", [KT, Wp], DT, kind="ExternalInput")
    bg3t = nc.dram_tensor("bg3t", [TT, W], F32, kind="ExternalInput")

    bandsD = nc.dram_tensor("bands", [128, 10, 128], DT, kind="ExternalInput")
    biasD = nc.dram_tensor("biasv", [128, 1], F32, kind="ExternalInput")

    ODT = F16 if out16 else F32
    outm = nc.dram_tensor("outm", [B, TR, W], ODT, kind="ExternalOutput")
    outt = nc.dram_tensor("outt", [BH, TT, W], ODT, kind="ExternalOutput")

    chunks = []
    c = 0
    while c < W:
        chunks.append((c, min(c + CH, W)))
        c += CH

    with tile.TileContext(nc) as tc, ExitStack() as ctx:
        const_pool = ctx.enter_context(tc.tile_pool(name="const", bufs=1))
        rpool = ctx.enter_context(tc.tile_pool(name="rsb", bufs=1))
        xpool = ctx.enter_context(tc.tile_pool(name="xin", bufs=14))
        hpool = ctx.enter_context(tc.tile_pool(name="hin", bufs=1))
        opool = ctx.enter_context(tc.tile_pool(name="osb", bufs=12))
        ipsum = ctx.enter_context(tc.tile_pool(name="ips", bufs=4, space="PSUM"))

        # constants
        bsb = const_pool.tile([128, 10, 128], DT)
        nc.sync.dma_start(out=bsb[:], in_=bandsD[:])
        bias_sb = const_pool.tile([128, 1], F32)
        nc.sync.dma_start(out=bias_sb[:], in_=biasD[:])

        def conv_mms(psum, xt, ci, K, M, first, last):
            """3 dx matmuls per column chunk for conv ci on tile xt."""
            for (ca, cb) in chunks:
                for dxi in (1, 0, 2):
                    nc.tensor.matmul(
                        psum[0:M, ca:cb],
                        bsb[0:K, ci * 3 + dxi, 0:M],
                        xt[0:K, ca + dxi:cb + dxi],
                        start=(first and dxi == 1),
                        stop=(last and dxi == 2),
                    )

        def residual(h_src, c_src, bg_src, K, M, tag):
            """R band = conv(H0,W9) + conv(C0,W10) + bias + bgate3."""
            ht = hpool.tile([128, Wp], DT, tag="ht" + tag)
            ct = hpool.tile([128, Wp], DT, tag="ct" + tag)
            bgt = hpool.tile([128, W], F32, tag="bgt" + tag)
            nc.sync.dma_start(out=ht[0:K, :], in_=h_src)
            nc.sync.dma_start(out=ct[0:K, :], in_=c_src)
            nc.sync.dma_start(out=bgt[0:M, :], in_=bg_src)
            psum = ipsum.tile([128, W], F32, tag="ips")
            conv_mms(psum, ht, 1, K, M, True, False)
            conv_mms(psum, ct, 2, K, M, False, True)
            R = rpool.tile([128, W], DT, tag="R" + tag)
            nc.vector.scalar_tensor_tensor(
                out=R[0:M, :], in0=psum[0:M, :], scalar=bias_sb[0:M, :],
                in1=bgt[0:M, :],
                op0=mybir.AluOpType.add, op1=mybir.AluOpType.add,
            )
            return R

        def image_conv(x_src, K, M):
            xt = xpool.tile([128, Wp], DT, tag="xt")
            nc.sync.dma_start(out=xt[0:K, :], in_=x_src)
            psum = ipsum.tile([128, W], F32, tag="ips")
            conv_mms(psum, xt, 0, K, M, True, True)
            return psum

        def image_finish(psum, out_dst, R, M):
            ot = opool.tile([128, W], F32, tag="ot")
            nc.vector.tensor_add(out=ot[0:M, :], in0=psum[0:M, :], in1=R[0:M, :])
            if out16:
                o16 = opool.tile([128, W], F16, tag="o16")
                nc.scalar.activation(
                    o16[0:M, :], ot[0:M, :], mybir.ActivationFunctionType.Sigmoid,
                )
                nc.gpsimd.dma_start(out=out_dst, in_=o16[0:M, :])
            else:
                nc.scalar.activation(
                    ot[0:M, :], ot[0:M, :], mybir.ActivationFunctionType.Sigmoid,
                )
                nc.gpsimd.dma_start(out=out_dst, in_=ot[0:M, :])

        # image 0's convs first (PE can start as soon as x lands), then the
        # residuals, then the rest of the stream.
        ps0 = image_conv(xm[0], 128, TR)
        Rm = residual(h0m[:], c0m[:], bg3m[:], 128, TR, "m")
        Rt = residual(h0t[:], c0t[:], bg3t[:], KT, TT, "t")
        image_finish(ps0, outm[0], Rm, TR)
        image_finish(image_conv(xt4[0], KT, TT), outt[0], Rt, TT)

        for img in range(1, B):
            image_finish(image_conv(xm[img], 128, TR), outm[img], Rm, TR)
            if img % (B // BH) == 0:  # interleave a home-image tail every 8th
                g = img // (B // BH)
                if g < BH:
                    image_finish(image_conv(xt4[g], KT, TT), outt[g], Rt, TT)

    nc.compile()
    return nc


_NC_CACHE = {}


def _get_nc(B, H, W, dt_mode="fp16", out16=False, n_cores=N_CORES):
    key = (B, H, W, dt_mode, out16, n_cores)
    if key not in _NC_CACHE:
        _NC_CACHE[key] = _build_nc(B, H, W, dt_mode, out16, n_cores)
    return _NC_CACHE[key]


def _make_inmaps(x, H0, C0, Wconv, bconv, bgate, n_cores, dt_mode="fp16"):
    B = x.shape[0]
    H, W = x.shape[2], x.shape[3]
    BH = B // n_cores
    TT = H - TR * n_cores
    KT = TT + 2

    ndt = np.float16 if dt_mode == "fp16" else np.float32
    xp = np.pad(np.asarray(x, ndt).reshape(B, H, W), ((0, 0), (1, 1), (1, 1)))
    h0p = np.pad(np.asarray(H0, ndt)[0, 0], 1)
    c0p = np.pad(np.asarray(C0, ndt)[0, 0], 1)
    bg3 = np.ascontiguousarray(np.asarray(bgate, np.float32)[3])
    Wc = np.asarray(Wconv, np.float32)
    bands = _build_bands([Wc[8, 0, 0], Wc[9, 0, 0], Wc[10, 0, 0]]).astype(ndt)
    bc = np.asarray(bconv, np.float32)
    bias = np.full((128, 1), bc[8] + bc[9] + bc[10], np.float32)

    maps = []
    for c in range(n_cores):
        r0 = TR * c
        maps.append({
            "xm": np.ascontiguousarray(xp[:, r0:r0 + 128, :]),
            "h0m": np.ascontiguousarray(h0p[r0:r0 + 128, :]),
            "c0m": np.ascontiguousarray(c0p[r0:r0 + 128, :]),
            "bg3m": np.ascontiguousarray(bg3[r0:r0 + TR, :]),
            "xt4": np.ascontiguousarray(xp[c * BH:(c + 1) * BH, H - TT:H + 2, :]),
            "h0t": np.ascontiguousarray(h0p[H - TT:H + 2, :]),
            "c0t": np.ascontiguousarray(c0p[H - TT:H + 2, :]),
            "bg3t": np.ascontiguousarray(bg3[H - TT:, :]),
            "bands": bands, "biasv": bias,
        })
    return maps


def kernel(x, H0, C0, Wconv, bconv, bgate):
    B, _, H, W = x.shape
    dt_mode = os.environ.get("CONV_DT", "fp16")
    out16 = os.environ.get("CONV_OUT16", "1") == "1"
    nc = _get_nc(B, H, W, dt_mode, out16)
    in_maps = _make_inmaps(x, H0, C0, Wconv, bconv, bgate, N_CORES, dt_mode)
    trace = os.environ.get("CONV_TRACE", "") == "1"
    res = run_bass_kernel_spmd(nc, in_maps, list(range(N_CORES)), trace=trace)
    if trace:
        kernel.last_exec_time_ns = res.exec_time_ns
        kernel.last_results = res

    out = np.empty((B, H, W), np.float32)
    BH = B // N_CORES
    TT = H - TR * N_CORES
    for c in range(N_CORES):
        r0 = TR * c
        out[:, r0:r0 + TR, :] = res.results[c]["outm"].astype(np.float32)
        if TT:
            out[c * BH:(c + 1) * BH, H - TT:, :] = res.results[c]["outt"].astype(np.float32)
    return out.reshape(B, 1, H, W)
